# revision 44
# baseline (speedup 1.0000x reference)
"""Delphi dense transformer (B=2,T=1024,D=768,V=32768,L=4,H=12) on 8 TRN2 cores.

Sharding: 8-way token parallelism for the trunk + vocab-sharded lm_head.
Core c (g = c//4 batch, r = c%4) owns query blocks jA=r, jB=7-r (128 tokens
each) of batch g -- this balances causal attention exactly (9 kv-tile units
per core).  Per layer, each batch group of 4 cores AllGathers packed
K (feature-major) + V (token-major) in bf16; the final hidden states are
AllGathered over all 8 cores for the vocab-sharded tied lm_head.

Numerics: bf16 matmuls with fp32 PSUM accumulation, fp32 residual stream and
LN statistics.  LayerNorm scale `w` is folded host-side into the following
weight matrices (exact); all bias terms in the reference setup are zero
(asserted here).  Softmax runs without max-subtraction (scores are O(1))
using a host-built additive mask; row sums come from a ones-column appended
to V during the P@V matmul.
"""
import math
import sys
from contextlib import ExitStack

import numpy as np

sys.path.insert(0, "/opt/trn_rl_repo")

import ml_dtypes  # noqa: E402
import concourse.bass as bass  # noqa: E402
import concourse.tile as tile  # noqa: E402
from concourse import bacc, mybir  # noqa: E402
from concourse.bass_utils import run_bass_kernel_spmd  # noqa: E402
from concourse.masks import make_identity  # noqa: E402

BF16 = mybir.dt.bfloat16
F32 = mybir.dt.float32
NPBF16 = ml_dtypes.bfloat16

B, T, D, V, L, H = 2, 1024, 768, 32768, 4, 12
HD = D // H          # 64
NCORE = 8
TPC = 256            # tokens per core (2 blocks of 128)
DK = D // 128        # 6 feature tiles
VS = V // NCORE      # 4096 vocab rows per core
KV_V = 2 * 128 * 780  # v part: [slot, token, 12*(64+1)] with ones col
KV_K = D * TPC         # k part: [token-part, slot, feat] consumer-contiguous
KV_CAT = KV_V + KV_K
SL_V = KV_V // 2       # per-slot v segment
SL_K = KV_K // 2       # per-slot k segment
SL_CAT = SL_V + SL_K   # per-slot packed kv segment [v | k]

NEG = -10000.0

# block b of a batch lives on group-rank R(b), slot s(b) (0: first 128 rows)
RANK_OF = [b if b < 4 else 7 - b for b in range(8)]
SLOT_OF = [0 if b < 4 else 1 for b in range(8)]

_NC_CACHE = {}


def _build_nc(debug_taps=False, reps=1, fake_coll=False, skip=()):
    key = (debug_taps, reps, fake_coll, tuple(skip))
    if key in _NC_CACHE:
        return _NC_CACHE[key]
    nc = bacc.Bacc(None, num_devices=NCORE)

    x_tok = nc.dram_tensor("x_tok", [TPC, D], F32, kind="ExternalInput")
    sfm = nc.dram_tensor("sfm", [D // 2, TPC], BF16, kind="ExternalInput")
    cfm = nc.dram_tensor("cfm", [D // 2, TPC], BF16, kind="ExternalInput")
    bias_t = nc.dram_tensor("bias_t", [T, TPC], BF16, kind="ExternalInput")
    wae_s = nc.dram_tensor("wae_s", [D // 2, D], BF16, kind="ExternalInput")
    wae_c = nc.dram_tensor("wae_c", [D // 2, D], BF16, kind="ExternalInput")
    wqk = nc.dram_tensor("wqk", [L, D, 2 * D], BF16, kind="ExternalInput")
    wv = nc.dram_tensor("wv", [L, D, D], BF16, kind="ExternalInput")
    wproj = nc.dram_tensor("wproj", [L, D, D], BF16, kind="ExternalInput")
    wfc = nc.dram_tensor("wfc", [L, D, 4 * D], BF16, kind="ExternalInput")
    wfc2 = nc.dram_tensor("wfc2", [L, 4 * D, D], BF16, kind="ExternalInput")
    wlm = nc.dram_tensor("wlm", [D, VS], BF16, kind="ExternalInput")

    logits = nc.dram_tensor("logits", [NCORE * TPC, VS], F32,
                            kind="ExternalOutput")
    if debug_taps:
        dbg = nc.dram_tensor("dbg", [L + 1, TPC, D], F32, kind="ExternalOutput")
        dbg_kv = nc.dram_tensor("dbg_kv", [4 * KV_CAT], BF16,
                                kind="ExternalOutput")
        dbg_q = nc.dram_tensor("dbg_q", [128, DK, TPC], BF16,
                               kind="ExternalOutput")
        dbg_y = nc.dram_tensor("dbg_y", [128, DK, TPC], BF16,
                               kind="ExternalOutput")

    kv_cat = nc.dram_tensor("kv_cat", [2 * SL_CAT], BF16)
    kv_all = nc.dram_tensor("kv_all", [2 * 4 * SL_CAT], BF16)
    warm_in = nc.dram_tensor("warm_in", [128], BF16)
    warm_out = nc.dram_tensor("warm_out", [4 * 128], BF16)
    xh_loc = nc.dram_tensor("xh_loc", [2 * SL_K], BF16)
    xh_all = nc.dram_tensor("xh_all", [2 * NCORE * SL_K], BF16,
                            addr_space="Shared")

    with tile.TileContext(nc) as tc, ExitStack() as ctx:
        const = ctx.enter_context(tc.tile_pool(name="const", bufs=1))

        ident = const.tile([128, 128], BF16)
        make_identity(nc, ident)
        eps_t = const.tile([128, 1], F32)
        nc.vector.memset(eps_t[:], 1e-5)

        def _rep_body(rep):
            with ExitStack() as rctx:
                persist = rctx.enter_context(
                    tc.tile_pool(name=f"persist{rep}", bufs=1))
                work = rctx.enter_context(
                    tc.tile_pool(name=f"work{rep}", bufs=4))
                lmw = rctx.enter_context(tc.tile_pool(name=f"lmw{rep}", bufs=1))
                trunk = rctx.enter_context(ExitStack())
                zp = trunk.enter_context(tc.tile_pool(name=f"zp{rep}", bufs=2))
                gp = trunk.enter_context(tc.tile_pool(name=f"gp{rep}", bufs=1))
                wp = trunk.enter_context(tc.tile_pool(name=f"wp{rep}", bufs=2))
                wp4 = trunk.enter_context(tc.tile_pool(name=f"wp4{rep}", bufs=3))
                kvp = trunk.enter_context(tc.tile_pool(name=f"kvp{rep}", bufs=1))
                ptp = trunk.enter_context(tc.tile_pool(name=f"ptp{rep}", bufs=2))
                wqkp = trunk.enter_context(
                    tc.tile_pool(name=f"wqkp{rep}", bufs=2))
                x_sb = persist.tile([128, 2, D], F32)
                nc.sync.dma_start(out=x_sb[:],
                                  in_=x_tok[:].rearrange("(s p) d -> p s d", p=128))
                bias_sb = persist.tile([128, 8, TPC], BF16)
                nc.sync.dma_start(out=bias_sb[:],
                                  in_=bias_t[:].rearrange("(b p) q -> p b q", p=128))
                if not fake_coll:
                    # tiny dummy AllGather: absorbs the one-time collective
                    # rendezvous barrier (~35us) during the embedding phase
                    nc.gpsimd.collective_compute(
                        "AllGather", mybir.AluOpType.bypass,
                        replica_groups=[[0, 1, 2, 3], [4, 5, 6, 7]],
                        ins=[warm_in[:]], outs=[warm_out[:]])

                # ---- embedding: x += sin/cos(ang) @ wae (interleave folded host-side)
                sf_sb = work.tile([128, 3, TPC], BF16)
                nc.sync.dma_start(out=sf_sb[:],
                                  in_=sfm[:].rearrange("(a p) t -> p a t", p=128))
                cf_sb = work.tile([128, 3, TPC], BF16)
                nc.sync.dma_start(out=cf_sb[:],
                                  in_=cfm[:].rearrange("(a p) t -> p a t", p=128))
                ws_sb = wp.tile([128, 3, D], BF16, tag="wae")
                nc.sync.dma_start(out=ws_sb[:],
                                  in_=wae_s[:].rearrange("(a p) d -> p a d", p=128))
                wc_sb = wp.tile([128, 3, D], BF16, tag="wae")
                nc.sync.dma_start(out=wc_sb[:],
                                  in_=wae_c[:].rearrange("(a p) d -> p a d", p=128))
                with tc.tile_pool(name="pemb", bufs=2, space="PSUM") as pemb:
                    for s in range(2):
                        for noff, nsz in ((0, 512), (512, 256)):
                            pe = pemb.tile([128, 512], F32)
                            for a in range(3):
                                nc.tensor.matmul(pe[:, :nsz],
                                                 sf_sb[:, a, s * 128:(s + 1) * 128],
                                                 ws_sb[:, a, noff:noff + nsz],
                                                 start=(a == 0), stop=False)
                            for a in range(3):
                                nc.tensor.matmul(pe[:, :nsz],
                                                 cf_sb[:, a, s * 128:(s + 1) * 128],
                                                 wc_sb[:, a, noff:noff + nsz],
                                                 start=False, stop=(a == 2))
                            nc.vector.tensor_add(x_sb[:, s, noff:noff + nsz],
                                                 x_sb[:, s, noff:noff + nsz],
                                                 pe[:, :nsz])

                if debug_taps:
                    nc.sync.dma_start(
                        out=dbg[0].rearrange("(s p) d -> p s d", p=128), in_=x_sb[:])

                def layer_norm_half(dst_bf16, s):
                    if 'ln' in skip:
                        nc.scalar.copy(dst_bf16[:, s, :], x_sb[:, s, :])
                        return
                    if True:
                        stats = work.tile([128, 3, 6], F32, tag="lnstats")
                        for i in range(3):
                            nc.vector.bn_stats(out=stats[:, i, :],
                                               in_=x_sb[:, s, i * 256:(i + 1) * 256])
                        mv = work.tile([128, 2], F32, tag="lnmv")
                        nc.vector.bn_aggr(out=mv[:], in_=stats[:])
                        rstd = work.tile([128, 1], F32, tag="lnrstd")
                        nc.scalar.activation(rstd[:], mv[:, 1:2],
                                             mybir.ActivationFunctionType.Sqrt,
                                             bias=eps_t[:])
                        rec = work.tile([128, 1], F32, tag="lnrec")
                        nc.vector.reciprocal(rec[:], rstd[:])
                        nc.vector.tensor_scalar(
                            dst_bf16[:, s, :], x_sb[:, s, :],
                            scalar1=mv[:, 0:1], scalar2=rec[:],
                            op0=mybir.AluOpType.subtract,
                            op1=mybir.AluOpType.mult)

                def layer_norm(dst_bf16):
                    for s in range(2):
                        layer_norm_half(dst_bf16, s)

                def transpose_to_fm(src_bf16, dst_fm):
                    """[128, 2, D] token-major -> [128, DK, 256] feature-major."""
                    with tc.tile_pool(name="ptr", bufs=3, space="PSUM") as ptr:
                        for s in range(2):
                            for a in range(DK):
                                pt_ = ptr.tile([128, 128], BF16)
                                nc.tensor.transpose(
                                    pt_[:], src_bf16[:, s, a * 128:(a + 1) * 128],
                                    ident[:])
                                if a % 2 == 0:
                                    nc.scalar.copy(
                                        dst_fm[:, a, s * 128:(s + 1) * 128], pt_[:])
                                else:
                                    nc.vector.tensor_copy(
                                        dst_fm[:, a, s * 128:(s + 1) * 128], pt_[:])

                wlm_ks = []

                for layer in range(L):
                    if layer >= L - 2 and 'lm' not in skip:
                        # prefetch lm-head weights on the ACT DMA queue,
                        # 3 tiles each during layers 2 and 3, so the
                        # transfers hide in weight-stream slack
                        for k in range((layer - (L - 2)) * 3,
                                       (layer - (L - 2)) * 3 + 3):
                            wlm_k = lmw.tile([128, VS], BF16, tag=f"wlm{k}")
                            nc.scalar.dma_start(
                                out=wlm_k[:],
                                in_=wlm[k * 128:(k + 1) * 128, :])
                            wlm_ks.append(wlm_k)

                    # ---- LN1 + transpose to feature-major
                    z_sb = zp.tile([128, 2, D], BF16, tag="z")
                    layer_norm(z_sb)
                    z_fm = zp.tile([128, DK, TPC], BF16, tag="zfm")
                    transpose_to_fm(z_sb, z_fm)

                    # ---- k,q feature-major (K cols first in wqk): the K
                    # AllGather launches as soon as the 6 K tiles are done,
                    # overlapping the remaining Q tiles + all of V.
                    q_fm = gp.tile([128, DK, TPC], BF16, tag="qfm")
                    k_loc = zp.tile([128, 2, DK, 128], BF16, tag="kvout")

                    def _k_write():
                        for sl_ in range(2):
                            nc.sync.dma_start(
                                out=bass.AP(tensor=kv_cat[:].tensor,
                                            offset=sl_ * SL_CAT + SL_V,
                                            ap=[[DK * 128, 128], [1, DK * 128]]),
                                in_=k_loc[:, sl_])

                    def _qk_chunk(ch, pqk):
                        wt = wqkp.tile([128, DK, 512], BF16, tag="wqk")
                        nc.sync.dma_start(
                            out=wt[:],
                            in_=wqk[layer, :, ch * 512:(ch + 1) * 512]
                            .rearrange("(k p) m -> p k m", p=128))
                        for mm in range(4):
                            m = ch * 4 + mm
                            pq = pqk.tile([128, TPC], F32)
                            for k in range(DK):
                                nc.tensor.matmul(
                                    pq[:], wt[:, k, mm * 128:(mm + 1) * 128],
                                    z_fm[:, k, :],
                                    start=(k == 0), stop=(k == DK - 1))
                            if m >= DK:
                                nc.vector.tensor_scalar_mul(
                                    q_fm[:, m - DK, :], pq[:], 1.0 / 8.0)
                            else:
                                nc.vector.tensor_copy(k_loc[:, 0, m, :],
                                                      pq[:, 0:128])
                                nc.vector.tensor_copy(k_loc[:, 1, m, :],
                                                      pq[:, 128:TPC])
                            if m == DK - 1:
                                _k_write()

                    pqk_stack = ExitStack()
                    pqk = pqk_stack.enter_context(
                        tc.tile_pool(name="pqk", bufs=4, space="PSUM"))
                    _qk_chunk(0, pqk)
                    _qk_chunk(1, pqk)

                    # ---- v token-major with on-chip [12x(64+1)] interleave
                    v_loc = zp.tile([128, 2, H * (HD + 1)], BF16, tag="kvout")
                    wvt = wp.tile([128, DK, D], BF16, tag="wsq")
                    nc.sync.dma_start(
                        out=wvt[:],
                        in_=wv[layer].rearrange("(k p) n -> p k n", p=128))
                    def _kv_gather(sl_):
                        # per-slot AllGather: slot 0 lands ~20us earlier than
                        # a merged gather would, and attention half 0 only
                        # needs slot-0 KV, so it starts while slot 1 flies
                        if fake_coll:
                            for R in range(4):
                                nc.gpsimd.dma_start(
                                    out=kv_all[(sl_ * 4 + R) * SL_CAT:
                                               (sl_ * 4 + R + 1) * SL_CAT],
                                    in_=kv_cat[sl_ * SL_CAT:(sl_ + 1) * SL_CAT])
                        else:
                            nc.gpsimd.collective_compute(
                                "AllGather", mybir.AluOpType.bypass,
                                replica_groups=[[0, 1, 2, 3], [4, 5, 6, 7]],
                                ins=[kv_cat[sl_ * SL_CAT:(sl_ + 1) * SL_CAT]],
                                outs=[kv_all[sl_ * 4 * SL_CAT:
                                             (sl_ + 1) * 4 * SL_CAT]])

                    with tc.tile_pool(name="pv", bufs=2, space="PSUM") as pv:
                        for s in range(2):
                            vv = v_loc[:, s, :].rearrange("p (h c) -> p h c",
                                                          c=HD + 1)
                            nc.vector.memset(vv[:, :, HD:HD + 1], 1.0)
                            for ch in range(3):  # 4 heads per 256-col chunk
                                pvt = pv.tile([128, 256], F32)
                                for k in range(DK):
                                    nc.tensor.matmul(
                                        pvt[:],
                                        z_fm[:, k, s * 128:(s + 1) * 128],
                                        wvt[:, k, ch * 256:(ch + 1) * 256],
                                        start=(k == 0), stop=(k == DK - 1))
                                if ch % 2 == 0:
                                    nc.vector.tensor_copy(
                                        vv[:, 4 * ch:4 * ch + 4, 0:HD],
                                        pvt[:].rearrange("p (h c) -> p h c", c=HD))
                                else:
                                    nc.scalar.copy(
                                        vv[:, 4 * ch:4 * ch + 4, 0:HD],
                                        pvt[:].rearrange("p (h c) -> p h c", c=HD))
                            nc.sync.dma_start(
                                out=bass.AP(tensor=kv_cat[:].tensor,
                                            offset=s * SL_CAT,
                                            ap=[[780, 128], [1, 780]]),
                                in_=v_loc[:, s, :])
                            _kv_gather(s)

                    _qk_chunk(2, pqk)
                    pqk_stack.close()

                    if debug_taps and layer == 0:
                        nc.sync.dma_start(out=dbg_kv[:2 * 4 * SL_CAT], in_=kv_all[:])
                        nc.sync.dma_start(out=dbg_q[:], in_=q_fm[:])
                    if debug_taps and layer == 0:
                        post_attn_dbg = True
                    else:
                        post_attn_dbg = False

                    # ---- load gathered K then V, one batched DMA per
                    # slot.  Queue placement matters: a gather-dependent DMA
                    # parks its whole queue, so slot 0 rides the ACT queue
                    # (first exp needs it anyway) and slot 1 rides the Pool
                    # queue right behind the slot-1 collective; the sync
                    # queue carries only the weight stream and never stalls.
                    k_g, v_g = [], []
                    for sl, eng in ((0, nc.scalar), (1, nc.gpsimd)):
                        kt = kvp.tile([128, 4, D], BF16, tag=f"kg{sl}")
                        eng.dma_start(out=kt[:], in_=bass.AP(
                            tensor=kv_all[:].tensor,
                            offset=sl * 4 * SL_CAT + SL_V,
                            ap=[[D, 128], [SL_CAT, 4], [1, D]]))
                        k_g.append(kt)
                        vt = kvp.tile([128, 4, H * (HD + 1)], BF16,
                                      tag=f"vg{sl}")
                        eng.dma_start(out=vt[:], in_=bass.AP(
                            tensor=kv_all[:].tensor,
                            offset=sl * 4 * SL_CAT,
                            ap=[[H * (HD + 1), 128], [SL_CAT, 4],
                                [1, H * (HD + 1)]]))
                        v_g.append(vt)


                    wpt = wp.tile([128, DK, D], BF16, tag="wsq")
                    nc.sync.dma_start(
                        out=wpt[:],
                        in_=wproj[layer].rearrange("(k p) n -> p k n", p=128))


                    # ---- attention, head-paired: scores for heads (2hh, 2hh+1)
                    # issue adjacently on partition rows 0-63 / 64-127 (distinct
                    # row groups -> concurrent on HW); AV is transposed
                    # (out[q, 64+1] = pt^T @ V) so the denominator lands as a
                    # per-partition column and normalize is a plain
                    # tensor_scalar on DVE.  Half 0's MLP chunks are emitted
                    # interleaved with half 1's pairs so the in-order PE
                    # stream has ready work during the softmax ACT phases.
                    y_sb = zp.tile([128, 2, D], BF16, tag="z")
                    y_fm = gp.tile([128, DK, TPC], BF16, tag="yfm")
                    z2_fm = zp.tile([128, DK, TPC], BF16, tag="zfm")
                    z2_sb = zp.tile([128, 2, D], BF16, tag="z")
                    g_fm = gp.tile([128, 24, TPC], BF16, tag="gfm")
                    with tc.tile_pool(name="pml", bufs=2, space="PSUM") as pml, \
                         ExitStack() as att_stack:
                        pstp = att_stack.enter_context(
                            tc.tile_pool(name="pat", bufs=2, space="PSUM"))
                        ppvp = att_stack.enter_context(
                            tc.tile_pool(name="ppv", bufs=2, space="PSUM"))

                        def attn_scores(half, hh):
                            hb = 4 if half == 0 else 8
                            qs = half * 128
                            pt01 = ptp.tile([128, 2, 8, 128], BF16, tag="pt")
                            pt0 = pt01[:, 0]
                            pt1 = pt01[:, 1]
                            for g4 in range(hb // 4):
                                # both heads' 4 score tiles in one 2-bank
                                # psum tile -> a single exp covers the pair
                                ps01 = pstp.tile([128, 1024], F32, tag="pst")
                                for bb_ in range(4):
                                    b = g4 * 4 + bb_
                                    for po, off in ((0, 0), (64, 512)):
                                        nc.tensor.matmul(
                                            ps01[:, off + bb_ * 128:
                                                 off + (bb_ + 1) * 128],
                                            k_g[SLOT_OF[b]][po:po + 64, RANK_OF[b],
                                                            hh * 128:(hh + 1) * 128],
                                            q_fm[po:po + 64, hh, qs:qs + 128],
                                            start=(bb_ == 0), stop=(bb_ == 3),
                                            skip_group_check=True)
                                nc.scalar.activation(
                                    pt01[:, :, g4 * 4:(g4 + 1) * 4, :],
                                    ps01[:].rearrange("p (h b i) -> p h b i",
                                                      h=2, b=4),
                                    mybir.ActivationFunctionType.Exp)
                                for pt in (pt0, pt1):
                                    nc.vector.tensor_mul(
                                        pt[:, g4 * 4:(g4 + 1) * 4, :],
                                        pt[:, g4 * 4:(g4 + 1) * 4, :],
                                        bias_sb[:, g4 * 4:(g4 + 1) * 4, qs:qs + 128])
                            return pt0, pt1

                        def attn_av(half, hh, pt0, pt1):
                            hb = 4 if half == 0 else 8
                            h0, h1 = 2 * hh, 2 * hh + 1
                            # AV pair shares one PSUM bank: h0 at cols 0:65
                            # (its start clears the bank), h1 at 128:193
                            # relying on per-element has_written
                            pv01 = ppvp.tile([128, 512], F32, tag="ppv")
                            for b in range(hb):
                                nc.tensor.matmul(
                                    pv01[:, 0:HD + 1],
                                    pt0[:, b, :],
                                    v_g[SLOT_OF[b]][:, RANK_OF[b],
                                                    h0 * (HD + 1):(h0 + 1) * (HD + 1)],
                                    start=(b == 0), stop=False,
                                    skip_group_check=True)
                                nc.tensor.matmul(
                                    pv01[:, 128:128 + HD + 1],
                                    pt1[:, b, :],
                                    v_g[SLOT_OF[b]][:, RANK_OF[b],
                                                    h1 * (HD + 1):(h1 + 1) * (HD + 1)],
                                    start=False, stop=(b == hb - 1),
                                    skip_group_check=True)
                            for off, h in ((0, h0), (128, h1)):
                                rec = work.tile([128, 1], F32, tag="srec")
                                nc.vector.reciprocal(
                                    rec[:], pv01[:, off + HD:off + HD + 1])
                                nc.vector.tensor_scalar_mul(
                                    y_sb[:, half, h * HD:(h + 1) * HD],
                                    pv01[:, off:off + HD], rec[:])

                        def _load_fc_w(ch):
                            wt = wp4.tile([128, DK, D], BF16, tag="wmlp")
                            nc.sync.dma_start(
                                out=wt[:],
                                in_=wfc[layer, :, ch * D:(ch + 1) * D]
                                .rearrange("(k p) m -> p k m", p=128))
                            return wt

                        def _load_fc2_w(ch):
                            wt = wp4.tile([128, DK, D], BF16, tag="wmlp")
                            nc.sync.dma_start(
                                out=wt[:],
                                in_=wfc2[layer, ch * D:(ch + 1) * D, :]
                                .rearrange("(k p) n -> p k n", p=128))
                            return wt

                        def y_transpose(half):
                            qs = half * 128
                            for a in range(DK):
                                ptt = pml.tile([128, 128], BF16, tag="mm")
                                nc.tensor.transpose(
                                    ptt[:], y_sb[:, half, a * 128:(a + 1) * 128],
                                    ident[:])
                                if a % 2 == 0:
                                    nc.scalar.copy(y_fm[:, a, qs:qs + 128], ptt[:])
                                else:
                                    nc.vector.tensor_copy(y_fm[:, a, qs:qs + 128],
                                                          ptt[:])
                            if post_attn_dbg and half == 1:
                                nc.sync.dma_start(out=dbg_y[:], in_=y_fm[:])

                        def proj_chunk(half, noff, nsz):
                            qs = half * 128
                            pp = pml.tile([128, 512], F32, tag="mm")
                            for k in range(DK):
                                nc.tensor.matmul(
                                    pp[:, :nsz],
                                    y_fm[:, k, qs:qs + 128],
                                    wpt[:, k, noff:noff + nsz],
                                    start=(k == 0), stop=(k == DK - 1))
                            nc.vector.tensor_add(x_sb[:, half, noff:noff + nsz],
                                                 x_sb[:, half, noff:noff + nsz],
                                                 pp[:, :nsz])

                        def ln2_chunk(half):
                            qs = half * 128
                            layer_norm_half(z2_sb, half)
                            for a in range(DK):
                                ptt = pml.tile([128, 128], BF16, tag="mm")
                                nc.tensor.transpose(
                                    ptt[:], z2_sb[:, half, a * 128:(a + 1) * 128],
                                    ident[:])
                                if a % 2 == 0:
                                    nc.scalar.copy(z2_fm[:, a, qs:qs + 128], ptt[:])
                                else:
                                    nc.vector.tensor_copy(z2_fm[:, a, qs:qs + 128],
                                                          ptt[:])

                        def fc_chunk(half, ch, wt):
                            qs = half * 128
                            for mm_ in range(6):
                                m = ch * 6 + mm_
                                pg = pml.tile([128, 128], F32, tag="mm")
                                for k in range(DK):
                                    nc.tensor.matmul(
                                        pg[:], wt[:, k, mm_ * 128:(mm_ + 1) * 128],
                                        z2_fm[:, k, qs:qs + 128],
                                        start=(k == 0), stop=(k == DK - 1))
                                nc.scalar.activation(
                                    g_fm[:, m, qs:qs + 128], pg[:],
                                    mybir.ActivationFunctionType.Gelu_apprx_tanh)

                        mlp = 'mlp' not in skip
                        fcw = [_load_fc_w(ch) for ch in range(3)] if mlp else []

                        # half-0 MLP chunks interleaved into half-1 pairs;
                        # within a pair the chunk sits between the score MMs
                        # and the exp-dependent AV MMs so the in-order PE
                        # queue always has ready work while ACT runs exp
                        # keep ACT-free work (transpose/proj) in the
                        # interleave; LN2 (sqrt) and fc (gelu) run after
                        # attention so the ACT table set switches only
                        # sqrt -> exp -> sqrt -> gelu per layer (~1.3us per
                        # table load)
                        chunks0 = [lambda: y_transpose(0),
                                   lambda: proj_chunk(0, 0, 512),
                                   lambda: proj_chunk(0, 512, 256)]
                        if 'attn' in skip:
                            nc.vector.memset(y_sb[:], 0.0)
                        else:
                            # half 0: software-pipeline scores(hh+1) ahead of
                            # AV(hh) so exp latency is hidden
                            pts = attn_scores(0, 0)
                            for hh in range(1, 6):
                                nxt = attn_scores(0, hh)
                                attn_av(0, hh - 1, *pts)
                                pts = nxt
                            attn_av(0, 5, *pts)
                        for hh in range(6):
                            if 'attn' not in skip:
                                pts = attn_scores(1, hh)
                            if hh < len(chunks0):
                                chunks0[hh]()
                            if 'attn' not in skip:
                                attn_av(1, hh, *pts)
                        for fn in chunks0[6:]:
                            fn()

                        # half-1 MLP + remaining fc chunks; weight-slot reuse
                        # order: fc(1,c0) frees t0 -> load c3; fc(1,c1) frees
                        # t1 -> load fc2w0; etc.
                        y_transpose(1)
                        proj_chunk(1, 0, 512)
                        proj_chunk(1, 512, 256)
                        fc2w = []
                        if mlp:
                            ln2_chunk(0)
                            ln2_chunk(1)
                            fc_chunk(0, 0, fcw[0])
                            fc_chunk(0, 1, fcw[1])
                            fc_chunk(0, 2, fcw[2])
                            fc_chunk(1, 0, fcw[0])
                            fcw.append(_load_fc_w(3))
                            fc_chunk(1, 1, fcw[1])
                            fc2w.append(_load_fc2_w(0))
                            fc_chunk(1, 2, fcw[2])
                            fc2w.append(_load_fc2_w(1))
                            fc_chunk(0, 3, fcw[3])
                            fc_chunk(1, 3, fcw[3])
                            fc2w.append(_load_fc2_w(2))

                        att_stack.close()
                        if mlp:
                            # ---- fc2 (token-major out, both halves) +
                            # residual.  3 banks: 512-wide chunk per half in
                            # its own bank, both 256-wide tails packed into
                            # one bank via per-element has_written (the s0
                            # start clears the bank before s1's first write)
                            with tc.tile_pool(name="pf2", bufs=1,
                                              space="PSUM") as pf2:
                                pf2_a = pf2.tile([128, 512], F32, tag="f2a")
                                pf2_b = pf2.tile([128, 512], F32, tag="f2b")
                                pf2_c = pf2.tile([128, 512], F32, tag="f2c")
                                outs = [(pf2_a[:, 0:512], 0, 0, 512, True),
                                        (pf2_c[:, 0:256], 0, 512, 256, True),
                                        (pf2_b[:, 0:512], 1, 0, 512, True),
                                        (pf2_c[:, 256:512], 1, 512, 256, False)]
                                for ch in range(4):
                                    if ch == 3:
                                        fc2w.append(_load_fc2_w(3))
                                    wt = fc2w[ch]
                                    for kk in range(DK):
                                        K24 = ch * DK + kk
                                        for po, s, noff, nsz, first in outs:
                                            nc.tensor.matmul(
                                                po,
                                                g_fm[:, K24, s * 128:(s + 1) * 128],
                                                wt[:, kk, noff:noff + nsz],
                                                start=(K24 == 0 and first),
                                                stop=(K24 == 23),
                                                skip_group_check=True)
                                for po, s, noff, nsz, first in outs:
                                    nc.vector.tensor_add(
                                        x_sb[:, s, noff:noff + nsz],
                                        x_sb[:, s, noff:noff + nsz], po)

                    if debug_taps:
                        nc.sync.dma_start(
                            out=dbg[layer + 1].rearrange("(s p) d -> p s d", p=128),
                            in_=x_sb[:])

                # ---- final LN + transpose + per-slot AllGather of hidden
                # states: slot 0 gathers while slot 1 is normalized, and the
                # slot-1 gather hides under the slot-0 lm matmuls
                z3_sb = zp.tile([128, 2, D], BF16, tag="z")
                z3_fm = zp.tile([128, DK, TPC], BF16, tag="zfm")
                for s in range(2):
                    layer_norm_half(z3_sb, s)
                    with tc.tile_pool(name=f"ptr3{s}", bufs=3,
                                      space="PSUM") as ptr3:
                        for a in range(DK):
                            pt_ = ptr3.tile([128, 128], BF16)
                            nc.tensor.transpose(
                                pt_[:], z3_sb[:, s, a * 128:(a + 1) * 128],
                                ident[:])
                            if a % 2 == 0:
                                nc.scalar.copy(
                                    z3_fm[:, a, s * 128:(s + 1) * 128], pt_[:])
                            else:
                                nc.vector.tensor_copy(
                                    z3_fm[:, a, s * 128:(s + 1) * 128], pt_[:])
                    nc.sync.dma_start(
                        out=bass.AP(tensor=xh_loc[:].tensor, offset=s * SL_K,
                                    ap=[[DK * 128, 128], [1, DK * 128]]),
                        in_=z3_fm[:, :, s * 128:(s + 1) * 128])
                    if fake_coll:
                        for R in range(NCORE):
                            nc.gpsimd.dma_start(
                                out=xh_all[(s * NCORE + R) * SL_K:
                                           (s * NCORE + R + 1) * SL_K],
                                in_=xh_loc[s * SL_K:(s + 1) * SL_K])
                    else:
                        nc.gpsimd.collective_compute(
                            "AllGather", mybir.AluOpType.bypass,
                            replica_groups=[[0, 1, 2, 3, 4, 5, 6, 7]],
                            ins=[xh_loc[s * SL_K:(s + 1) * SL_K]],
                            outs=[xh_all[s * NCORE * SL_K:
                                         (s + 1) * NCORE * SL_K]])

                # ---- lm head: logits[tok, vs] = xh^T @ Wlm, vocab-sharded;
                # s-major so slot-0 rows compute during the slot-1 gather
                trunk.close()
                if 'lm' in skip:
                    return
                obp = rctx.enter_context(tc.tile_pool(name=f"obp{rep}", bufs=2))
                with tc.tile_pool(name="plm", bufs=2, space="PSUM") as plm, \
                     tc.tile_pool(name="xtp", bufs=2, space="SBUF") as xtp:
                    for s in range(2):
                        xt_s = xtp.tile([128, NCORE, DK * 128], BF16, tag="xt")
                        nc.sync.dma_start(out=xt_s[:], in_=bass.AP(
                            tensor=xh_all[:].tensor, offset=s * NCORE * SL_K,
                            ap=[[DK * 128, 128], [SL_K, NCORE],
                                [1, DK * 128]]))
                        for R in range(NCORE):
                            gq, rq = R // 4, R % 4
                            blk = rq if s == 0 else 7 - rq
                            row = gq * T + blk * 128
                            ob = obp.tile([128, 4096], F32, tag="ob")
                            for half in range(2):
                                pl = plm.tile([128, 2048], F32)
                                for k in range(DK):
                                    for nb in range(4):
                                        nc.tensor.matmul(
                                            pl[:, nb * 512:(nb + 1) * 512],
                                            xt_s[:, R, k * 128:(k + 1) * 128],
                                            wlm_ks[k][:,
                                                   half * 2048 + nb * 512:
                                                   half * 2048 + (nb + 1) * 512],
                                            start=(k == 0), stop=(k == DK - 1))
                                nc.vector.tensor_copy(ob[:, half * 2048:half * 2048 + 1024],
                                                      pl[:, 0:1024])
                                nc.scalar.copy(ob[:, half * 2048 + 1024:(half + 1) * 2048],
                                               pl[:, 1024:2048])
                            nc.sync.dma_start(out=logits[row:row + 128, :], in_=ob[:])

        for rep in range(reps):
            _rep_body(rep)

    nc.compile()
    _NC_CACHE[key] = nc
    return nc


def _prep_in_maps(inputs):
    idx = np.asarray(inputs["idx"])
    age = np.asarray(inputs["age"], np.float32)
    wte = np.asarray(inputs["wte"], np.float32)
    wae_w = np.asarray(inputs["wae_w"], np.float32)
    ln1_w = np.asarray(inputs["ln1_w"], np.float32)
    ln2_w = np.asarray(inputs["ln2_w"], np.float32)
    lnf_w = np.asarray(inputs["lnf_w"], np.float32)
    attn_w = np.asarray(inputs["attn_w"], np.float32)
    proj_w = np.asarray(inputs["proj_w"], np.float32)
    fc_w = np.asarray(inputs["fc_w"], np.float32)
    fc2_w = np.asarray(inputs["fc2_w"], np.float32)
    for nm in ("ln1_b", "ln2_b", "lnf_b", "attn_b", "proj_b", "fc_b", "fc2_b"):
        assert not np.any(np.asarray(inputs[nm])), f"{nm} != 0 unsupported"

    bf = lambda a: np.ascontiguousarray(a).astype(NPBF16)

    # replicated weights (LN scale folded in)
    wqk_l, wv_l, wproj_l, wfc_l, wfc2_l = [], [], [], [], []
    for l in range(L):
        aw = attn_w[l] * ln1_w[l][None, :]
        # K columns first so the K AllGather can launch while Q computes
        wqk_l.append(np.concatenate([aw[D:2 * D].T, aw[:D].T], axis=1))
        wv_l.append(aw[2 * D:].T)
        wproj_l.append(proj_w[l].T)
        wfc_l.append((fc_w[l] * ln2_w[l][None, :]).T)
        wfc2_l.append(fc2_w[l].T)
    wqk_a = bf(np.stack(wqk_l))
    wv_a = bf(np.stack(wv_l))
    wproj_a = bf(np.stack(wproj_l))
    wfc_a = bf(np.stack(wfc_l))
    wfc2_a = bf(np.stack(wfc2_l))
    wae_s_a = bf(wae_w[:, 0::2].T)   # [384, 768]
    wae_c_a = bf(wae_w[:, 1::2].T)
    wlm_full = wte * lnf_w[None, :]  # [V, D]

    div = np.exp(np.arange(0, D, 2, dtype=np.float32) *
                 (-math.log(10000.0) / D))
    valid = idx > 0
    karange = np.arange(T)

    in_maps = []
    for c in range(NCORE):
        g, r = c // 4, c % 4
        jA, jB = r, 7 - r
        tok_idx = np.concatenate([np.arange(jA * 128, (jA + 1) * 128),
                                  np.arange(jB * 128, (jB + 1) * 128)])
        x_tok = wte[np.asarray(idx[g])[tok_idx]].astype(np.float32)
        ang = div[:, None] * (age[g][tok_idx][None, :] / 365.25)  # [384, 256]
        vq = valid[g][tok_idx]
        vk = valid[g]
        keep = (karange[:, None] <= tok_idx[None, :]) & (
            (vq[None, :] & vk[:, None]) |
            (~vq[None, :] & (karange[:, None] == tok_idx[None, :])))
        bias_tc = keep.astype(np.float32).astype(NPBF16)
        in_maps.append({
            "x_tok": x_tok,
            "sfm": bf(np.sin(ang)),
            "cfm": bf(np.cos(ang)),
            "bias_t": bias_tc,
            "wae_s": wae_s_a, "wae_c": wae_c_a,
            "wqk": wqk_a, "wv": wv_a, "wproj": wproj_a,
            "wfc": wfc_a, "wfc2": wfc2_a,
            "wlm": bf(wlm_full[c * VS:(c + 1) * VS].T),
        })
    return in_maps


last_results = None


def kernel(debug_taps=False, _trace=False, _tmpdir=None, **inputs):
    global last_results
    nc = _build_nc(debug_taps)
    in_maps = _prep_in_maps(inputs)
    res = run_bass_kernel_spmd(nc, in_maps, core_ids=list(range(NCORE)),
                               trace=_trace, tmpdir=_tmpdir)
    last_results = res
    out = np.empty((B, T, V), np.float32)
    for c in range(NCORE):
        out[:, :, c * VS:(c + 1) * VS] = \
            res.results[c]["logits"].reshape(B, T, VS)
    if debug_taps:
        return out, [r["dbg"] for r in res.results], \
            [r["dbg_kv"] for r in res.results], \
            [r["dbg_q"] for r in res.results], \
            [r["dbg_y"] for r in res.results]
    return out



# revision 45
# speedup vs baseline: 1.0008x; 1.0008x over previous
"""Delphi dense transformer (B=2,T=1024,D=768,V=32768,L=4,H=12) on 8 TRN2 cores.

Sharding: 8-way token parallelism for the trunk + vocab-sharded lm_head.
Core c (g = c//4 batch, r = c%4) owns query blocks jA=r, jB=7-r (128 tokens
each) of batch g -- this balances causal attention exactly (9 kv-tile units
per core).  Per layer, each batch group of 4 cores AllGathers packed
K (feature-major) + V (token-major) in bf16; the final hidden states are
AllGathered over all 8 cores for the vocab-sharded tied lm_head.

Numerics: bf16 matmuls with fp32 PSUM accumulation, fp32 residual stream and
LN statistics.  LayerNorm scale `w` is folded host-side into the following
weight matrices (exact); all bias terms in the reference setup are zero
(asserted here).  Softmax runs without max-subtraction (scores are O(1))
using a host-built additive mask; row sums come from a ones-column appended
to V during the P@V matmul.
"""
import math
import sys
from contextlib import ExitStack

import numpy as np

sys.path.insert(0, "/opt/trn_rl_repo")

import ml_dtypes  # noqa: E402
import concourse.bass as bass  # noqa: E402
import concourse.tile as tile  # noqa: E402
from concourse import bacc, mybir  # noqa: E402
from concourse.bass_utils import run_bass_kernel_spmd  # noqa: E402
from concourse.masks import make_identity  # noqa: E402

BF16 = mybir.dt.bfloat16
F32 = mybir.dt.float32
NPBF16 = ml_dtypes.bfloat16

B, T, D, V, L, H = 2, 1024, 768, 32768, 4, 12
HD = D // H          # 64
NCORE = 8
TPC = 256            # tokens per core (2 blocks of 128)
DK = D // 128        # 6 feature tiles
VS = V // NCORE      # 4096 vocab rows per core
KV_V = 2 * 128 * 780  # v part: [slot, token, 12*(64+1)] with ones col
KV_K = D * TPC         # k part: [token-part, slot, feat] consumer-contiguous
KV_CAT = KV_V + KV_K
SL_V = KV_V // 2       # per-slot v segment
SL_K = KV_K // 2       # per-slot k segment
SL_CAT = SL_V + SL_K   # per-slot packed kv segment [v | k]

NEG = -10000.0

# block b of a batch lives on group-rank R(b), slot s(b) (0: first 128 rows)
RANK_OF = [b if b < 4 else 7 - b for b in range(8)]
SLOT_OF = [0 if b < 4 else 1 for b in range(8)]

_NC_CACHE = {}


def _build_nc(debug_taps=False, reps=1, fake_coll=False, skip=()):
    key = (debug_taps, reps, fake_coll, tuple(skip))
    if key in _NC_CACHE:
        return _NC_CACHE[key]
    nc = bacc.Bacc(None, num_devices=NCORE)

    x_tok = nc.dram_tensor("x_tok", [TPC, D], F32, kind="ExternalInput")
    sfm = nc.dram_tensor("sfm", [D // 2, TPC], BF16, kind="ExternalInput")
    cfm = nc.dram_tensor("cfm", [D // 2, TPC], BF16, kind="ExternalInput")
    bias_t = nc.dram_tensor("bias_t", [T, TPC], BF16, kind="ExternalInput")
    wae_s = nc.dram_tensor("wae_s", [D // 2, D], BF16, kind="ExternalInput")
    wae_c = nc.dram_tensor("wae_c", [D // 2, D], BF16, kind="ExternalInput")
    wqk = nc.dram_tensor("wqk", [L, D, 2 * D], BF16, kind="ExternalInput")
    wv = nc.dram_tensor("wv", [L, D, D], BF16, kind="ExternalInput")
    wproj = nc.dram_tensor("wproj", [L, D, D], BF16, kind="ExternalInput")
    wfc = nc.dram_tensor("wfc", [L, D, 4 * D], BF16, kind="ExternalInput")
    wfc2 = nc.dram_tensor("wfc2", [L, 4 * D, D], BF16, kind="ExternalInput")
    wlm = nc.dram_tensor("wlm", [D, VS], BF16, kind="ExternalInput")

    logits = nc.dram_tensor("logits", [NCORE * TPC, VS], F32,
                            kind="ExternalOutput")
    if debug_taps:
        dbg = nc.dram_tensor("dbg", [L + 1, TPC, D], F32, kind="ExternalOutput")
        dbg_kv = nc.dram_tensor("dbg_kv", [4 * KV_CAT], BF16,
                                kind="ExternalOutput")
        dbg_q = nc.dram_tensor("dbg_q", [128, DK, TPC], BF16,
                               kind="ExternalOutput")
        dbg_y = nc.dram_tensor("dbg_y", [128, DK, TPC], BF16,
                               kind="ExternalOutput")

    kv_cat = nc.dram_tensor("kv_cat", [2 * SL_CAT], BF16)
    kv_all = nc.dram_tensor("kv_all", [2 * 4 * SL_CAT], BF16)
    warm_in = nc.dram_tensor("warm_in", [128], BF16)
    warm_out = nc.dram_tensor("warm_out", [4 * 128], BF16)
    xh_loc = nc.dram_tensor("xh_loc", [2 * SL_K], BF16)
    xh_all = nc.dram_tensor("xh_all", [2 * NCORE * SL_K], BF16,
                            addr_space="Shared")

    with tile.TileContext(nc) as tc, ExitStack() as ctx:
        const = ctx.enter_context(tc.tile_pool(name="const", bufs=1))

        ident = const.tile([128, 128], BF16)
        make_identity(nc, ident)
        eps_t = const.tile([128, 1], F32)
        nc.vector.memset(eps_t[:], 1e-5)

        def _rep_body(rep):
            with ExitStack() as rctx:
                persist = rctx.enter_context(
                    tc.tile_pool(name=f"persist{rep}", bufs=1))
                work = rctx.enter_context(
                    tc.tile_pool(name=f"work{rep}", bufs=4))
                lmw = rctx.enter_context(tc.tile_pool(name=f"lmw{rep}", bufs=1))
                trunk = rctx.enter_context(ExitStack())
                zp = trunk.enter_context(tc.tile_pool(name=f"zp{rep}", bufs=2))
                gp = trunk.enter_context(tc.tile_pool(name=f"gp{rep}", bufs=1))
                wp = trunk.enter_context(tc.tile_pool(name=f"wp{rep}", bufs=2))
                wp4 = trunk.enter_context(tc.tile_pool(name=f"wp4{rep}", bufs=3))
                kvp = trunk.enter_context(tc.tile_pool(name=f"kvp{rep}", bufs=1))
                ptp = trunk.enter_context(tc.tile_pool(name=f"ptp{rep}", bufs=2))
                wqkp = trunk.enter_context(
                    tc.tile_pool(name=f"wqkp{rep}", bufs=2))
                x_sb = persist.tile([128, 2, D], F32)
                nc.sync.dma_start(out=x_sb[:],
                                  in_=x_tok[:].rearrange("(s p) d -> p s d", p=128))
                bias_sb = persist.tile([128, 8, TPC], BF16)
                nc.sync.dma_start(out=bias_sb[:],
                                  in_=bias_t[:].rearrange("(b p) q -> p b q", p=128))
                if not fake_coll:
                    # tiny dummy AllGather: absorbs the one-time collective
                    # rendezvous barrier (~35us) during the embedding phase
                    nc.gpsimd.collective_compute(
                        "AllGather", mybir.AluOpType.bypass,
                        replica_groups=[[0, 1, 2, 3], [4, 5, 6, 7]],
                        ins=[warm_in[:]], outs=[warm_out[:]])

                # ---- embedding: x += sin/cos(ang) @ wae (interleave folded host-side)
                sf_sb = work.tile([128, 3, TPC], BF16)
                nc.sync.dma_start(out=sf_sb[:],
                                  in_=sfm[:].rearrange("(a p) t -> p a t", p=128))
                cf_sb = work.tile([128, 3, TPC], BF16)
                nc.sync.dma_start(out=cf_sb[:],
                                  in_=cfm[:].rearrange("(a p) t -> p a t", p=128))
                ws_sb = wp.tile([128, 3, D], BF16, tag="wae")
                nc.sync.dma_start(out=ws_sb[:],
                                  in_=wae_s[:].rearrange("(a p) d -> p a d", p=128))
                wc_sb = wp.tile([128, 3, D], BF16, tag="wae")
                nc.sync.dma_start(out=wc_sb[:],
                                  in_=wae_c[:].rearrange("(a p) d -> p a d", p=128))
                with tc.tile_pool(name="pemb", bufs=2, space="PSUM") as pemb:
                    for s in range(2):
                        for noff, nsz in ((0, 512), (512, 256)):
                            pe = pemb.tile([128, 512], F32)
                            for a in range(3):
                                nc.tensor.matmul(pe[:, :nsz],
                                                 sf_sb[:, a, s * 128:(s + 1) * 128],
                                                 ws_sb[:, a, noff:noff + nsz],
                                                 start=(a == 0), stop=False)
                            for a in range(3):
                                nc.tensor.matmul(pe[:, :nsz],
                                                 cf_sb[:, a, s * 128:(s + 1) * 128],
                                                 wc_sb[:, a, noff:noff + nsz],
                                                 start=False, stop=(a == 2))
                            nc.vector.tensor_add(x_sb[:, s, noff:noff + nsz],
                                                 x_sb[:, s, noff:noff + nsz],
                                                 pe[:, :nsz])

                if debug_taps:
                    nc.sync.dma_start(
                        out=dbg[0].rearrange("(s p) d -> p s d", p=128), in_=x_sb[:])

                def layer_norm_half(dst_bf16, s):
                    if 'ln' in skip:
                        nc.scalar.copy(dst_bf16[:, s, :], x_sb[:, s, :])
                        return
                    if True:
                        stats = work.tile([128, 3, 6], F32, tag="lnstats")
                        for i in range(3):
                            nc.vector.bn_stats(out=stats[:, i, :],
                                               in_=x_sb[:, s, i * 256:(i + 1) * 256])
                        mv = work.tile([128, 2], F32, tag="lnmv")
                        nc.vector.bn_aggr(out=mv[:], in_=stats[:])
                        rstd = work.tile([128, 1], F32, tag="lnrstd")
                        nc.scalar.activation(rstd[:], mv[:, 1:2],
                                             mybir.ActivationFunctionType.Sqrt,
                                             bias=eps_t[:])
                        rec = work.tile([128, 1], F32, tag="lnrec")
                        nc.vector.reciprocal(rec[:], rstd[:])
                        nc.vector.tensor_scalar(
                            dst_bf16[:, s, :], x_sb[:, s, :],
                            scalar1=mv[:, 0:1], scalar2=rec[:],
                            op0=mybir.AluOpType.subtract,
                            op1=mybir.AluOpType.mult)

                def layer_norm(dst_bf16):
                    for s in range(2):
                        layer_norm_half(dst_bf16, s)

                def transpose_to_fm(src_bf16, dst_fm):
                    """[128, 2, D] token-major -> [128, DK, 256] feature-major."""
                    with tc.tile_pool(name="ptr", bufs=3, space="PSUM") as ptr:
                        for s in range(2):
                            for a in range(DK):
                                pt_ = ptr.tile([128, 128], BF16)
                                nc.tensor.transpose(
                                    pt_[:], src_bf16[:, s, a * 128:(a + 1) * 128],
                                    ident[:])
                                if a % 2 == 0:
                                    nc.scalar.copy(
                                        dst_fm[:, a, s * 128:(s + 1) * 128], pt_[:])
                                else:
                                    nc.vector.tensor_copy(
                                        dst_fm[:, a, s * 128:(s + 1) * 128], pt_[:])

                wlm_ks = []

                for layer in range(L):
                    if layer >= L - 2 and 'lm' not in skip:
                        # prefetch lm-head weights on the ACT DMA queue,
                        # 3 tiles each during layers 2 and 3, so the
                        # transfers hide in weight-stream slack
                        for k in range((layer - (L - 2)) * 3,
                                       (layer - (L - 2)) * 3 + 3):
                            wlm_k = lmw.tile([128, VS], BF16, tag=f"wlm{k}")
                            nc.scalar.dma_start(
                                out=wlm_k[:],
                                in_=wlm[k * 128:(k + 1) * 128, :])
                            wlm_ks.append(wlm_k)

                    # ---- LN1 + transpose to feature-major
                    z_sb = zp.tile([128, 2, D], BF16, tag="z")
                    layer_norm(z_sb)
                    z_fm = zp.tile([128, DK, TPC], BF16, tag="zfm")
                    transpose_to_fm(z_sb, z_fm)

                    # ---- k,q feature-major (K cols first in wqk): the K
                    # AllGather launches as soon as the 6 K tiles are done,
                    # overlapping the remaining Q tiles + all of V.
                    q_fm = gp.tile([128, DK, TPC], BF16, tag="qfm")
                    k_loc = zp.tile([128, 2, DK, 128], BF16, tag="kvout")

                    def _k_write():
                        for sl_ in range(2):
                            nc.sync.dma_start(
                                out=bass.AP(tensor=kv_cat[:].tensor,
                                            offset=sl_ * SL_CAT + SL_V,
                                            ap=[[DK * 128, 128], [1, DK * 128]]),
                                in_=k_loc[:, sl_])

                    def _qk_chunk(ch, pqk):
                        wt = wqkp.tile([128, DK, 512], BF16, tag="wqk")
                        nc.sync.dma_start(
                            out=wt[:],
                            in_=wqk[layer, :, ch * 512:(ch + 1) * 512]
                            .rearrange("(k p) m -> p k m", p=128))
                        for mm in range(4):
                            m = ch * 4 + mm
                            pq = pqk.tile([128, TPC], F32)
                            for k in range(DK):
                                nc.tensor.matmul(
                                    pq[:], wt[:, k, mm * 128:(mm + 1) * 128],
                                    z_fm[:, k, :],
                                    start=(k == 0), stop=(k == DK - 1))
                            if m >= DK:
                                nc.vector.tensor_scalar_mul(
                                    q_fm[:, m - DK, :], pq[:], 1.0 / 8.0)
                            else:
                                nc.vector.tensor_copy(k_loc[:, 0, m, :],
                                                      pq[:, 0:128])
                                nc.vector.tensor_copy(k_loc[:, 1, m, :],
                                                      pq[:, 128:TPC])
                            if m == DK - 1:
                                _k_write()

                    pqk_stack = ExitStack()
                    pqk = pqk_stack.enter_context(
                        tc.tile_pool(name="pqk", bufs=4, space="PSUM"))
                    _qk_chunk(0, pqk)
                    _qk_chunk(1, pqk)

                    # ---- v token-major with on-chip [12x(64+1)] interleave
                    v_loc = zp.tile([128, 2, H * (HD + 1)], BF16, tag="kvout")
                    wvt = wp.tile([128, DK, D], BF16, tag="wsq")
                    nc.sync.dma_start(
                        out=wvt[:],
                        in_=wv[layer].rearrange("(k p) n -> p k n", p=128))
                    def _kv_gather(sl_):
                        # per-slot AllGather: slot 0 lands ~20us earlier than
                        # a merged gather would, and attention half 0 only
                        # needs slot-0 KV, so it starts while slot 1 flies
                        if fake_coll:
                            for R in range(4):
                                nc.gpsimd.dma_start(
                                    out=kv_all[(sl_ * 4 + R) * SL_CAT:
                                               (sl_ * 4 + R + 1) * SL_CAT],
                                    in_=kv_cat[sl_ * SL_CAT:(sl_ + 1) * SL_CAT])
                        else:
                            nc.gpsimd.collective_compute(
                                "AllGather", mybir.AluOpType.bypass,
                                replica_groups=[[0, 1, 2, 3], [4, 5, 6, 7]],
                                ins=[kv_cat[sl_ * SL_CAT:(sl_ + 1) * SL_CAT]],
                                outs=[kv_all[sl_ * 4 * SL_CAT:
                                             (sl_ + 1) * 4 * SL_CAT]])

                    with tc.tile_pool(name="pv", bufs=2, space="PSUM") as pv:
                        for s in range(2):
                            vv = v_loc[:, s, :].rearrange("p (h c) -> p h c",
                                                          c=HD + 1)
                            nc.vector.memset(vv[:, :, HD:HD + 1], 1.0)
                            for ch in range(3):  # 4 heads per 256-col chunk
                                pvt = pv.tile([128, 256], F32)
                                for k in range(DK):
                                    nc.tensor.matmul(
                                        pvt[:],
                                        z_fm[:, k, s * 128:(s + 1) * 128],
                                        wvt[:, k, ch * 256:(ch + 1) * 256],
                                        start=(k == 0), stop=(k == DK - 1))
                                if ch % 2 == 0:
                                    nc.vector.tensor_copy(
                                        vv[:, 4 * ch:4 * ch + 4, 0:HD],
                                        pvt[:].rearrange("p (h c) -> p h c", c=HD))
                                else:
                                    nc.scalar.copy(
                                        vv[:, 4 * ch:4 * ch + 4, 0:HD],
                                        pvt[:].rearrange("p (h c) -> p h c", c=HD))
                            nc.sync.dma_start(
                                out=bass.AP(tensor=kv_cat[:].tensor,
                                            offset=s * SL_CAT,
                                            ap=[[780, 128], [1, 780]]),
                                in_=v_loc[:, s, :])
                            _kv_gather(s)

                    _qk_chunk(2, pqk)
                    pqk_stack.close()

                    if debug_taps and layer == 0:
                        nc.sync.dma_start(out=dbg_kv[:2 * 4 * SL_CAT], in_=kv_all[:])
                        nc.sync.dma_start(out=dbg_q[:], in_=q_fm[:])
                    if debug_taps and layer == 0:
                        post_attn_dbg = True
                    else:
                        post_attn_dbg = False

                    # ---- load gathered K then V, one batched DMA per
                    # slot.  Queue placement matters: a gather-dependent DMA
                    # parks its whole queue, so slot 0 rides the ACT queue
                    # (first exp needs it anyway) and slot 1 rides the Pool
                    # queue right behind the slot-1 collective; the sync
                    # queue carries only the weight stream and never stalls.
                    k_g, v_g = [], []
                    for sl, eng in ((0, nc.scalar), (1, nc.sync)):
                        kt = kvp.tile([128, 4, D], BF16, tag=f"kg{sl}")
                        eng.dma_start(out=kt[:], in_=bass.AP(
                            tensor=kv_all[:].tensor,
                            offset=sl * 4 * SL_CAT + SL_V,
                            ap=[[D, 128], [SL_CAT, 4], [1, D]]))
                        k_g.append(kt)
                        vt = kvp.tile([128, 4, H * (HD + 1)], BF16,
                                      tag=f"vg{sl}")
                        eng.dma_start(out=vt[:], in_=bass.AP(
                            tensor=kv_all[:].tensor,
                            offset=sl * 4 * SL_CAT,
                            ap=[[H * (HD + 1), 128], [SL_CAT, 4],
                                [1, H * (HD + 1)]]))
                        v_g.append(vt)


                    wpt = wp.tile([128, DK, D], BF16, tag="wsq")
                    nc.sync.dma_start(
                        out=wpt[:],
                        in_=wproj[layer].rearrange("(k p) n -> p k n", p=128))


                    # ---- attention, head-paired: scores for heads (2hh, 2hh+1)
                    # issue adjacently on partition rows 0-63 / 64-127 (distinct
                    # row groups -> concurrent on HW); AV is transposed
                    # (out[q, 64+1] = pt^T @ V) so the denominator lands as a
                    # per-partition column and normalize is a plain
                    # tensor_scalar on DVE.  Half 0's MLP chunks are emitted
                    # interleaved with half 1's pairs so the in-order PE
                    # stream has ready work during the softmax ACT phases.
                    y_sb = zp.tile([128, 2, D], BF16, tag="z")
                    y_fm = gp.tile([128, DK, TPC], BF16, tag="yfm")
                    z2_fm = zp.tile([128, DK, TPC], BF16, tag="zfm")
                    z2_sb = zp.tile([128, 2, D], BF16, tag="z")
                    g_fm = gp.tile([128, 24, TPC], BF16, tag="gfm")
                    with tc.tile_pool(name="pml", bufs=2, space="PSUM") as pml, \
                         ExitStack() as att_stack:
                        pstp = att_stack.enter_context(
                            tc.tile_pool(name="pat", bufs=2, space="PSUM"))
                        ppvp = att_stack.enter_context(
                            tc.tile_pool(name="ppv", bufs=2, space="PSUM"))

                        def attn_scores(half, hh):
                            hb = 4 if half == 0 else 8
                            qs = half * 128
                            pt01 = ptp.tile([128, 2, 8, 128], BF16, tag="pt")
                            pt0 = pt01[:, 0]
                            pt1 = pt01[:, 1]
                            for g4 in range(hb // 4):
                                # both heads' 4 score tiles in one 2-bank
                                # psum tile -> a single exp covers the pair
                                ps01 = pstp.tile([128, 1024], F32, tag="pst")
                                for bb_ in range(4):
                                    b = g4 * 4 + bb_
                                    for po, off in ((0, 0), (64, 512)):
                                        nc.tensor.matmul(
                                            ps01[:, off + bb_ * 128:
                                                 off + (bb_ + 1) * 128],
                                            k_g[SLOT_OF[b]][po:po + 64, RANK_OF[b],
                                                            hh * 128:(hh + 1) * 128],
                                            q_fm[po:po + 64, hh, qs:qs + 128],
                                            start=(bb_ == 0), stop=(bb_ == 3),
                                            skip_group_check=True)
                                nc.scalar.activation(
                                    pt01[:, :, g4 * 4:(g4 + 1) * 4, :],
                                    ps01[:].rearrange("p (h b i) -> p h b i",
                                                      h=2, b=4),
                                    mybir.ActivationFunctionType.Exp)
                                for pt in (pt0, pt1):
                                    nc.vector.tensor_mul(
                                        pt[:, g4 * 4:(g4 + 1) * 4, :],
                                        pt[:, g4 * 4:(g4 + 1) * 4, :],
                                        bias_sb[:, g4 * 4:(g4 + 1) * 4, qs:qs + 128])
                            return pt0, pt1

                        def attn_av(half, hh, pt0, pt1):
                            hb = 4 if half == 0 else 8
                            h0, h1 = 2 * hh, 2 * hh + 1
                            # AV pair shares one PSUM bank: h0 at cols 0:65
                            # (its start clears the bank), h1 at 128:193
                            # relying on per-element has_written
                            pv01 = ppvp.tile([128, 512], F32, tag="ppv")
                            for b in range(hb):
                                nc.tensor.matmul(
                                    pv01[:, 0:HD + 1],
                                    pt0[:, b, :],
                                    v_g[SLOT_OF[b]][:, RANK_OF[b],
                                                    h0 * (HD + 1):(h0 + 1) * (HD + 1)],
                                    start=(b == 0), stop=False,
                                    skip_group_check=True)
                                nc.tensor.matmul(
                                    pv01[:, 128:128 + HD + 1],
                                    pt1[:, b, :],
                                    v_g[SLOT_OF[b]][:, RANK_OF[b],
                                                    h1 * (HD + 1):(h1 + 1) * (HD + 1)],
                                    start=False, stop=(b == hb - 1),
                                    skip_group_check=True)
                            for off, h in ((0, h0), (128, h1)):
                                rec = work.tile([128, 1], F32, tag="srec")
                                nc.vector.reciprocal(
                                    rec[:], pv01[:, off + HD:off + HD + 1])
                                nc.vector.tensor_scalar_mul(
                                    y_sb[:, half, h * HD:(h + 1) * HD],
                                    pv01[:, off:off + HD], rec[:])

                        def _load_fc_w(ch):
                            wt = wp4.tile([128, DK, D], BF16, tag="wmlp")
                            nc.sync.dma_start(
                                out=wt[:],
                                in_=wfc[layer, :, ch * D:(ch + 1) * D]
                                .rearrange("(k p) m -> p k m", p=128))
                            return wt

                        def _load_fc2_w(ch):
                            wt = wp4.tile([128, DK, D], BF16, tag="wmlp")
                            nc.sync.dma_start(
                                out=wt[:],
                                in_=wfc2[layer, ch * D:(ch + 1) * D, :]
                                .rearrange("(k p) n -> p k n", p=128))
                            return wt

                        def y_transpose(half):
                            qs = half * 128
                            for a in range(DK):
                                ptt = pml.tile([128, 128], BF16, tag="mm")
                                nc.tensor.transpose(
                                    ptt[:], y_sb[:, half, a * 128:(a + 1) * 128],
                                    ident[:])
                                if a % 2 == 0:
                                    nc.scalar.copy(y_fm[:, a, qs:qs + 128], ptt[:])
                                else:
                                    nc.vector.tensor_copy(y_fm[:, a, qs:qs + 128],
                                                          ptt[:])
                            if post_attn_dbg and half == 1:
                                nc.sync.dma_start(out=dbg_y[:], in_=y_fm[:])

                        def proj_chunk(half, noff, nsz):
                            qs = half * 128
                            pp = pml.tile([128, 512], F32, tag="mm")
                            for k in range(DK):
                                nc.tensor.matmul(
                                    pp[:, :nsz],
                                    y_fm[:, k, qs:qs + 128],
                                    wpt[:, k, noff:noff + nsz],
                                    start=(k == 0), stop=(k == DK - 1))
                            nc.vector.tensor_add(x_sb[:, half, noff:noff + nsz],
                                                 x_sb[:, half, noff:noff + nsz],
                                                 pp[:, :nsz])

                        def ln2_chunk(half):
                            qs = half * 128
                            layer_norm_half(z2_sb, half)
                            for a in range(DK):
                                ptt = pml.tile([128, 128], BF16, tag="mm")
                                nc.tensor.transpose(
                                    ptt[:], z2_sb[:, half, a * 128:(a + 1) * 128],
                                    ident[:])
                                if a % 2 == 0:
                                    nc.scalar.copy(z2_fm[:, a, qs:qs + 128], ptt[:])
                                else:
                                    nc.vector.tensor_copy(z2_fm[:, a, qs:qs + 128],
                                                          ptt[:])

                        def fc_chunk(half, ch, wt):
                            qs = half * 128
                            for mm_ in range(6):
                                m = ch * 6 + mm_
                                pg = pml.tile([128, 128], F32, tag="mm")
                                for k in range(DK):
                                    nc.tensor.matmul(
                                        pg[:], wt[:, k, mm_ * 128:(mm_ + 1) * 128],
                                        z2_fm[:, k, qs:qs + 128],
                                        start=(k == 0), stop=(k == DK - 1))
                                nc.scalar.activation(
                                    g_fm[:, m, qs:qs + 128], pg[:],
                                    mybir.ActivationFunctionType.Gelu_apprx_tanh)

                        mlp = 'mlp' not in skip
                        fcw = [_load_fc_w(ch) for ch in range(3)] if mlp else []

                        # half-0 MLP chunks interleaved into half-1 pairs;
                        # within a pair the chunk sits between the score MMs
                        # and the exp-dependent AV MMs so the in-order PE
                        # queue always has ready work while ACT runs exp
                        # keep ACT-free work (transpose/proj) in the
                        # interleave; LN2 (sqrt) and fc (gelu) run after
                        # attention so the ACT table set switches only
                        # sqrt -> exp -> sqrt -> gelu per layer (~1.3us per
                        # table load)
                        chunks0 = [lambda: y_transpose(0),
                                   lambda: proj_chunk(0, 0, 512),
                                   lambda: proj_chunk(0, 512, 256)]
                        if 'attn' in skip:
                            nc.vector.memset(y_sb[:], 0.0)
                        else:
                            # half 0: software-pipeline scores(hh+1) ahead of
                            # AV(hh) so exp latency is hidden
                            pts = attn_scores(0, 0)
                            for hh in range(1, 6):
                                nxt = attn_scores(0, hh)
                                attn_av(0, hh - 1, *pts)
                                pts = nxt
                            attn_av(0, 5, *pts)
                        for hh in range(6):
                            if 'attn' not in skip:
                                pts = attn_scores(1, hh)
                            if hh < len(chunks0):
                                chunks0[hh]()
                            if 'attn' not in skip:
                                attn_av(1, hh, *pts)
                        for fn in chunks0[6:]:
                            fn()

                        # half-1 MLP + remaining fc chunks; weight-slot reuse
                        # order: fc(1,c0) frees t0 -> load c3; fc(1,c1) frees
                        # t1 -> load fc2w0; etc.
                        y_transpose(1)
                        proj_chunk(1, 0, 512)
                        proj_chunk(1, 512, 256)
                        fc2w = []
                        if mlp:
                            ln2_chunk(0)
                            ln2_chunk(1)
                            fc_chunk(0, 0, fcw[0])
                            fc_chunk(0, 1, fcw[1])
                            fc_chunk(0, 2, fcw[2])
                            fc_chunk(1, 0, fcw[0])
                            fcw.append(_load_fc_w(3))
                            fc_chunk(1, 1, fcw[1])
                            fc2w.append(_load_fc2_w(0))
                            fc_chunk(1, 2, fcw[2])
                            fc2w.append(_load_fc2_w(1))
                            fc_chunk(0, 3, fcw[3])
                            fc_chunk(1, 3, fcw[3])
                            fc2w.append(_load_fc2_w(2))

                        att_stack.close()
                        if mlp:
                            # ---- fc2 (token-major out, both halves) +
                            # residual.  3 banks: 512-wide chunk per half in
                            # its own bank, both 256-wide tails packed into
                            # one bank via per-element has_written (the s0
                            # start clears the bank before s1's first write)
                            with tc.tile_pool(name="pf2", bufs=1,
                                              space="PSUM") as pf2:
                                pf2_a = pf2.tile([128, 512], F32, tag="f2a")
                                pf2_b = pf2.tile([128, 512], F32, tag="f2b")
                                pf2_c = pf2.tile([128, 512], F32, tag="f2c")
                                outs = [(pf2_a[:, 0:512], 0, 0, 512, True),
                                        (pf2_c[:, 0:256], 0, 512, 256, True),
                                        (pf2_b[:, 0:512], 1, 0, 512, True),
                                        (pf2_c[:, 256:512], 1, 512, 256, False)]
                                for ch in range(4):
                                    if ch == 3:
                                        fc2w.append(_load_fc2_w(3))
                                    wt = fc2w[ch]
                                    for kk in range(DK):
                                        K24 = ch * DK + kk
                                        for po, s, noff, nsz, first in outs:
                                            nc.tensor.matmul(
                                                po,
                                                g_fm[:, K24, s * 128:(s + 1) * 128],
                                                wt[:, kk, noff:noff + nsz],
                                                start=(K24 == 0 and first),
                                                stop=(K24 == 23),
                                                skip_group_check=True)
                                for po, s, noff, nsz, first in outs:
                                    nc.vector.tensor_add(
                                        x_sb[:, s, noff:noff + nsz],
                                        x_sb[:, s, noff:noff + nsz], po)

                    if debug_taps:
                        nc.sync.dma_start(
                            out=dbg[layer + 1].rearrange("(s p) d -> p s d", p=128),
                            in_=x_sb[:])

                # ---- final LN + transpose + per-slot AllGather of hidden
                # states: slot 0 gathers while slot 1 is normalized, and the
                # slot-1 gather hides under the slot-0 lm matmuls
                z3_sb = zp.tile([128, 2, D], BF16, tag="z")
                z3_fm = zp.tile([128, DK, TPC], BF16, tag="zfm")
                for s in range(2):
                    layer_norm_half(z3_sb, s)
                    with tc.tile_pool(name=f"ptr3{s}", bufs=3,
                                      space="PSUM") as ptr3:
                        for a in range(DK):
                            pt_ = ptr3.tile([128, 128], BF16)
                            nc.tensor.transpose(
                                pt_[:], z3_sb[:, s, a * 128:(a + 1) * 128],
                                ident[:])
                            if a % 2 == 0:
                                nc.scalar.copy(
                                    z3_fm[:, a, s * 128:(s + 1) * 128], pt_[:])
                            else:
                                nc.vector.tensor_copy(
                                    z3_fm[:, a, s * 128:(s + 1) * 128], pt_[:])
                    nc.sync.dma_start(
                        out=bass.AP(tensor=xh_loc[:].tensor, offset=s * SL_K,
                                    ap=[[DK * 128, 128], [1, DK * 128]]),
                        in_=z3_fm[:, :, s * 128:(s + 1) * 128])
                    if fake_coll:
                        for R in range(NCORE):
                            nc.gpsimd.dma_start(
                                out=xh_all[(s * NCORE + R) * SL_K:
                                           (s * NCORE + R + 1) * SL_K],
                                in_=xh_loc[s * SL_K:(s + 1) * SL_K])
                    else:
                        nc.gpsimd.collective_compute(
                            "AllGather", mybir.AluOpType.bypass,
                            replica_groups=[[0, 1, 2, 3, 4, 5, 6, 7]],
                            ins=[xh_loc[s * SL_K:(s + 1) * SL_K]],
                            outs=[xh_all[s * NCORE * SL_K:
                                         (s + 1) * NCORE * SL_K]])

                # ---- lm head: logits[tok, vs] = xh^T @ Wlm, vocab-sharded;
                # s-major so slot-0 rows compute during the slot-1 gather
                trunk.close()
                if 'lm' in skip:
                    return
                obp = rctx.enter_context(tc.tile_pool(name=f"obp{rep}", bufs=2))
                with tc.tile_pool(name="plm", bufs=2, space="PSUM") as plm, \
                     tc.tile_pool(name="xtp", bufs=2, space="SBUF") as xtp:
                    for s in range(2):
                        xt_s = xtp.tile([128, NCORE, DK * 128], BF16, tag="xt")
                        nc.sync.dma_start(out=xt_s[:], in_=bass.AP(
                            tensor=xh_all[:].tensor, offset=s * NCORE * SL_K,
                            ap=[[DK * 128, 128], [SL_K, NCORE],
                                [1, DK * 128]]))
                        for R in range(NCORE):
                            gq, rq = R // 4, R % 4
                            blk = rq if s == 0 else 7 - rq
                            row = gq * T + blk * 128
                            ob = obp.tile([128, 4096], F32, tag="ob")
                            for half in range(2):
                                pl = plm.tile([128, 2048], F32)
                                for k in range(DK):
                                    for nb in range(4):
                                        nc.tensor.matmul(
                                            pl[:, nb * 512:(nb + 1) * 512],
                                            xt_s[:, R, k * 128:(k + 1) * 128],
                                            wlm_ks[k][:,
                                                   half * 2048 + nb * 512:
                                                   half * 2048 + (nb + 1) * 512],
                                            start=(k == 0), stop=(k == DK - 1))
                                nc.vector.tensor_copy(ob[:, half * 2048:half * 2048 + 1024],
                                                      pl[:, 0:1024])
                                nc.scalar.copy(ob[:, half * 2048 + 1024:(half + 1) * 2048],
                                               pl[:, 1024:2048])
                            nc.sync.dma_start(out=logits[row:row + 128, :], in_=ob[:])

        for rep in range(reps):
            _rep_body(rep)

    nc.compile()
    _NC_CACHE[key] = nc
    return nc


def _prep_in_maps(inputs):
    idx = np.asarray(inputs["idx"])
    age = np.asarray(inputs["age"], np.float32)
    wte = np.asarray(inputs["wte"], np.float32)
    wae_w = np.asarray(inputs["wae_w"], np.float32)
    ln1_w = np.asarray(inputs["ln1_w"], np.float32)
    ln2_w = np.asarray(inputs["ln2_w"], np.float32)
    lnf_w = np.asarray(inputs["lnf_w"], np.float32)
    attn_w = np.asarray(inputs["attn_w"], np.float32)
    proj_w = np.asarray(inputs["proj_w"], np.float32)
    fc_w = np.asarray(inputs["fc_w"], np.float32)
    fc2_w = np.asarray(inputs["fc2_w"], np.float32)
    for nm in ("ln1_b", "ln2_b", "lnf_b", "attn_b", "proj_b", "fc_b", "fc2_b"):
        assert not np.any(np.asarray(inputs[nm])), f"{nm} != 0 unsupported"

    bf = lambda a: np.ascontiguousarray(a).astype(NPBF16)

    # replicated weights (LN scale folded in)
    wqk_l, wv_l, wproj_l, wfc_l, wfc2_l = [], [], [], [], []
    for l in range(L):
        aw = attn_w[l] * ln1_w[l][None, :]
        # K columns first so the K AllGather can launch while Q computes
        wqk_l.append(np.concatenate([aw[D:2 * D].T, aw[:D].T], axis=1))
        wv_l.append(aw[2 * D:].T)
        wproj_l.append(proj_w[l].T)
        wfc_l.append((fc_w[l] * ln2_w[l][None, :]).T)
        wfc2_l.append(fc2_w[l].T)
    wqk_a = bf(np.stack(wqk_l))
    wv_a = bf(np.stack(wv_l))
    wproj_a = bf(np.stack(wproj_l))
    wfc_a = bf(np.stack(wfc_l))
    wfc2_a = bf(np.stack(wfc2_l))
    wae_s_a = bf(wae_w[:, 0::2].T)   # [384, 768]
    wae_c_a = bf(wae_w[:, 1::2].T)
    wlm_full = wte * lnf_w[None, :]  # [V, D]

    div = np.exp(np.arange(0, D, 2, dtype=np.float32) *
                 (-math.log(10000.0) / D))
    valid = idx > 0
    karange = np.arange(T)

    in_maps = []
    for c in range(NCORE):
        g, r = c // 4, c % 4
        jA, jB = r, 7 - r
        tok_idx = np.concatenate([np.arange(jA * 128, (jA + 1) * 128),
                                  np.arange(jB * 128, (jB + 1) * 128)])
        x_tok = wte[np.asarray(idx[g])[tok_idx]].astype(np.float32)
        ang = div[:, None] * (age[g][tok_idx][None, :] / 365.25)  # [384, 256]
        vq = valid[g][tok_idx]
        vk = valid[g]
        keep = (karange[:, None] <= tok_idx[None, :]) & (
            (vq[None, :] & vk[:, None]) |
            (~vq[None, :] & (karange[:, None] == tok_idx[None, :])))
        bias_tc = keep.astype(np.float32).astype(NPBF16)
        in_maps.append({
            "x_tok": x_tok,
            "sfm": bf(np.sin(ang)),
            "cfm": bf(np.cos(ang)),
            "bias_t": bias_tc,
            "wae_s": wae_s_a, "wae_c": wae_c_a,
            "wqk": wqk_a, "wv": wv_a, "wproj": wproj_a,
            "wfc": wfc_a, "wfc2": wfc2_a,
            "wlm": bf(wlm_full[c * VS:(c + 1) * VS].T),
        })
    return in_maps


last_results = None


def kernel(debug_taps=False, _trace=False, _tmpdir=None, **inputs):
    global last_results
    nc = _build_nc(debug_taps)
    in_maps = _prep_in_maps(inputs)
    res = run_bass_kernel_spmd(nc, in_maps, core_ids=list(range(NCORE)),
                               trace=_trace, tmpdir=_tmpdir)
    last_results = res
    out = np.empty((B, T, V), np.float32)
    for c in range(NCORE):
        out[:, :, c * VS:(c + 1) * VS] = \
            res.results[c]["logits"].reshape(B, T, VS)
    if debug_taps:
        return out, [r["dbg"] for r in res.results], \
            [r["dbg_kv"] for r in res.results], \
            [r["dbg_q"] for r in res.results], \
            [r["dbg_y"] for r in res.results]
    return out



# revision 46
# speedup vs baseline: 1.0198x; 1.0190x over previous
"""Delphi dense transformer (B=2,T=1024,D=768,V=32768,L=4,H=12) on 8 TRN2 cores.

Sharding: 8-way token parallelism for the trunk + vocab-sharded lm_head.
Core c (g = c//4 batch, r = c%4) owns query blocks jA=r, jB=7-r (128 tokens
each) of batch g -- this balances causal attention exactly (9 kv-tile units
per core).  Per layer, each batch group of 4 cores AllGathers packed
K (feature-major) + V (token-major) in bf16; the final hidden states are
AllGathered over all 8 cores for the vocab-sharded tied lm_head.

Numerics: bf16 matmuls with fp32 PSUM accumulation, fp32 residual stream and
LN statistics.  LayerNorm scale `w` is folded host-side into the following
weight matrices (exact); all bias terms in the reference setup are zero
(asserted here).  Softmax runs without max-subtraction (scores are O(1))
using a host-built additive mask; row sums come from a ones-column appended
to V during the P@V matmul.
"""
import math
import sys
from contextlib import ExitStack

import numpy as np

sys.path.insert(0, "/opt/trn_rl_repo")

import ml_dtypes  # noqa: E402
import concourse.bass as bass  # noqa: E402
import concourse.tile as tile  # noqa: E402
from concourse import bacc, mybir  # noqa: E402
from concourse.bass_utils import run_bass_kernel_spmd  # noqa: E402
from concourse.masks import make_identity  # noqa: E402

BF16 = mybir.dt.bfloat16
F32 = mybir.dt.float32
NPBF16 = ml_dtypes.bfloat16

B, T, D, V, L, H = 2, 1024, 768, 32768, 4, 12
HD = D // H          # 64
NCORE = 8
TPC = 256            # tokens per core (2 blocks of 128)
DK = D // 128        # 6 feature tiles
VS = V // NCORE      # 4096 vocab rows per core
KV_V = 2 * 128 * 780  # v part: [slot, token, 12*(64+1)] with ones col
KV_K = D * TPC         # k part: [token-part, slot, feat] consumer-contiguous
KV_CAT = KV_V + KV_K
SL_V = KV_V // 2       # per-slot v segment
SL_K = KV_K // 2       # per-slot k segment
SL_CAT = SL_V + SL_K   # per-slot packed kv segment [v | k]

NEG = -10000.0

# block b of a batch lives on group-rank R(b), slot s(b) (0: first 128 rows)
RANK_OF = [b if b < 4 else 7 - b for b in range(8)]
SLOT_OF = [0 if b < 4 else 1 for b in range(8)]

_NC_CACHE = {}


def _build_nc(debug_taps=False, reps=1, fake_coll=False, skip=()):
    key = (debug_taps, reps, fake_coll, tuple(skip))
    if key in _NC_CACHE:
        return _NC_CACHE[key]
    nc = bacc.Bacc(None, num_devices=NCORE)

    x_tok = nc.dram_tensor("x_tok", [TPC, D], F32, kind="ExternalInput")
    sfm = nc.dram_tensor("sfm", [D // 2, TPC], BF16, kind="ExternalInput")
    cfm = nc.dram_tensor("cfm", [D // 2, TPC], BF16, kind="ExternalInput")
    bias_t = nc.dram_tensor("bias_t", [T, TPC], BF16, kind="ExternalInput")
    wae_s = nc.dram_tensor("wae_s", [D // 2, D], BF16, kind="ExternalInput")
    wae_c = nc.dram_tensor("wae_c", [D // 2, D], BF16, kind="ExternalInput")
    wqk = nc.dram_tensor("wqk", [L, D, 2 * D], BF16, kind="ExternalInput")
    wv = nc.dram_tensor("wv", [L, D, D], BF16, kind="ExternalInput")
    wproj = nc.dram_tensor("wproj", [L, D, D], BF16, kind="ExternalInput")
    wfc = nc.dram_tensor("wfc", [L, D, 4 * D], BF16, kind="ExternalInput")
    wfc2 = nc.dram_tensor("wfc2", [L, 4 * D, D], BF16, kind="ExternalInput")
    wlm = nc.dram_tensor("wlm", [D, VS], BF16, kind="ExternalInput")

    logits = nc.dram_tensor("logits", [NCORE * TPC, VS], F32,
                            kind="ExternalOutput")
    if debug_taps:
        dbg = nc.dram_tensor("dbg", [L + 1, TPC, D], F32, kind="ExternalOutput")
        dbg_kv = nc.dram_tensor("dbg_kv", [4 * KV_CAT], BF16,
                                kind="ExternalOutput")
        dbg_q = nc.dram_tensor("dbg_q", [128, DK, TPC], BF16,
                               kind="ExternalOutput")
        dbg_y = nc.dram_tensor("dbg_y", [128, DK, TPC], BF16,
                               kind="ExternalOutput")

    kv_cat = nc.dram_tensor("kv_cat", [2 * SL_CAT], BF16)
    kv_all = nc.dram_tensor("kv_all", [2 * 4 * SL_CAT], BF16)
    warm_in = nc.dram_tensor("warm_in", [128], BF16)
    warm_out = nc.dram_tensor("warm_out", [4 * 128], BF16)
    xh_loc = nc.dram_tensor("xh_loc", [2 * SL_K], BF16)
    xh_all = nc.dram_tensor("xh_all", [2 * NCORE * SL_K], BF16,
                            addr_space="Shared")

    with tile.TileContext(nc) as tc, ExitStack() as ctx:
        const = ctx.enter_context(tc.tile_pool(name="const", bufs=1))

        ident = const.tile([128, 128], BF16)
        make_identity(nc, ident)
        eps_t = const.tile([128, 1], F32)
        nc.vector.memset(eps_t[:], 1e-5)

        def _rep_body(rep):
            with ExitStack() as rctx:
                persist = rctx.enter_context(
                    tc.tile_pool(name=f"persist{rep}", bufs=1))
                work = rctx.enter_context(
                    tc.tile_pool(name=f"work{rep}", bufs=4))
                lmw = rctx.enter_context(tc.tile_pool(name=f"lmw{rep}", bufs=1))
                trunk = rctx.enter_context(ExitStack())
                zp = trunk.enter_context(tc.tile_pool(name=f"zp{rep}", bufs=2))
                gp = trunk.enter_context(tc.tile_pool(name=f"gp{rep}", bufs=1))
                wp = trunk.enter_context(tc.tile_pool(name=f"wp{rep}", bufs=2))
                wp4 = trunk.enter_context(tc.tile_pool(name=f"wp4{rep}", bufs=3))
                kvp = trunk.enter_context(tc.tile_pool(name=f"kvp{rep}", bufs=1))
                ptp = trunk.enter_context(tc.tile_pool(name=f"ptp{rep}", bufs=2))
                wqkp = trunk.enter_context(
                    tc.tile_pool(name=f"wqkp{rep}", bufs=2))
                x_sb = persist.tile([128, 2, D], F32)
                nc.sync.dma_start(out=x_sb[:],
                                  in_=x_tok[:].rearrange("(s p) d -> p s d", p=128))
                bias_sb = persist.tile([128, 8, TPC], BF16)
                nc.sync.dma_start(out=bias_sb[:],
                                  in_=bias_t[:].rearrange("(b p) q -> p b q", p=128))
                if not fake_coll:
                    # tiny dummy AllGather: absorbs the one-time collective
                    # rendezvous barrier (~35us) during the embedding phase
                    nc.gpsimd.collective_compute(
                        "AllGather", mybir.AluOpType.bypass,
                        replica_groups=[[0, 1, 2, 3], [4, 5, 6, 7]],
                        ins=[warm_in[:]], outs=[warm_out[:]])

                # ---- embedding: x += sin/cos(ang) @ wae (interleave folded host-side)
                sf_sb = work.tile([128, 3, TPC], BF16)
                nc.sync.dma_start(out=sf_sb[:],
                                  in_=sfm[:].rearrange("(a p) t -> p a t", p=128))
                cf_sb = work.tile([128, 3, TPC], BF16)
                nc.sync.dma_start(out=cf_sb[:],
                                  in_=cfm[:].rearrange("(a p) t -> p a t", p=128))
                ws_sb = wp.tile([128, 3, D], BF16, tag="wae")
                nc.sync.dma_start(out=ws_sb[:],
                                  in_=wae_s[:].rearrange("(a p) d -> p a d", p=128))
                wc_sb = wp.tile([128, 3, D], BF16, tag="wae")
                nc.sync.dma_start(out=wc_sb[:],
                                  in_=wae_c[:].rearrange("(a p) d -> p a d", p=128))
                with tc.tile_pool(name="pemb", bufs=2, space="PSUM") as pemb:
                    for s in range(2):
                        for noff, nsz in ((0, 512), (512, 256)):
                            pe = pemb.tile([128, 512], F32)
                            for a in range(3):
                                nc.tensor.matmul(pe[:, :nsz],
                                                 sf_sb[:, a, s * 128:(s + 1) * 128],
                                                 ws_sb[:, a, noff:noff + nsz],
                                                 start=(a == 0), stop=False)
                            for a in range(3):
                                nc.tensor.matmul(pe[:, :nsz],
                                                 cf_sb[:, a, s * 128:(s + 1) * 128],
                                                 wc_sb[:, a, noff:noff + nsz],
                                                 start=False, stop=(a == 2))
                            nc.vector.tensor_add(x_sb[:, s, noff:noff + nsz],
                                                 x_sb[:, s, noff:noff + nsz],
                                                 pe[:, :nsz])

                if debug_taps:
                    nc.sync.dma_start(
                        out=dbg[0].rearrange("(s p) d -> p s d", p=128), in_=x_sb[:])

                def layer_norm_half(dst_bf16, s):
                    if 'ln' in skip:
                        nc.scalar.copy(dst_bf16[:, s, :], x_sb[:, s, :])
                        return
                    if True:
                        stats = work.tile([128, 3, 6], F32, tag="lnstats")
                        for i in range(3):
                            nc.vector.bn_stats(out=stats[:, i, :],
                                               in_=x_sb[:, s, i * 256:(i + 1) * 256])
                        mv = work.tile([128, 2], F32, tag="lnmv")
                        nc.vector.bn_aggr(out=mv[:], in_=stats[:])
                        rstd = work.tile([128, 1], F32, tag="lnrstd")
                        nc.scalar.activation(rstd[:], mv[:, 1:2],
                                             mybir.ActivationFunctionType.Sqrt,
                                             bias=eps_t[:])
                        rec = work.tile([128, 1], F32, tag="lnrec")
                        nc.vector.reciprocal(rec[:], rstd[:])
                        nc.vector.tensor_scalar(
                            dst_bf16[:, s, :], x_sb[:, s, :],
                            scalar1=mv[:, 0:1], scalar2=rec[:],
                            op0=mybir.AluOpType.subtract,
                            op1=mybir.AluOpType.mult)

                def layer_norm(dst_bf16):
                    for s in range(2):
                        layer_norm_half(dst_bf16, s)

                def transpose_to_fm(src_bf16, dst_fm):
                    """[128, 2, D] token-major -> [128, DK, 256] feature-major."""
                    with tc.tile_pool(name="ptr", bufs=3, space="PSUM") as ptr:
                        for s in range(2):
                            for a in range(DK):
                                pt_ = ptr.tile([128, 128], BF16)
                                nc.tensor.transpose(
                                    pt_[:], src_bf16[:, s, a * 128:(a + 1) * 128],
                                    ident[:])
                                if a % 2 == 0:
                                    nc.scalar.copy(
                                        dst_fm[:, a, s * 128:(s + 1) * 128], pt_[:])
                                else:
                                    nc.vector.tensor_copy(
                                        dst_fm[:, a, s * 128:(s + 1) * 128], pt_[:])

                wlm_ks = []

                for layer in range(L):
                    if layer >= L - 2 and 'lm' not in skip:
                        # prefetch lm-head weights on the ACT DMA queue,
                        # 3 tiles each during layers 2 and 3, so the
                        # transfers hide in weight-stream slack
                        for k in range((layer - (L - 2)) * 3,
                                       (layer - (L - 2)) * 3 + 3):
                            wlm_k = lmw.tile([128, VS], BF16, tag=f"wlm{k}")
                            nc.scalar.dma_start(
                                out=wlm_k[:],
                                in_=wlm[k * 128:(k + 1) * 128, :])
                            wlm_ks.append(wlm_k)

                    # ---- LN1 + transpose to feature-major
                    z_sb = zp.tile([128, 2, D], BF16, tag="z")
                    layer_norm(z_sb)
                    z_fm = zp.tile([128, DK, TPC], BF16, tag="zfm")
                    transpose_to_fm(z_sb, z_fm)

                    # ---- k,q feature-major (K cols first in wqk): the K
                    # AllGather launches as soon as the 6 K tiles are done,
                    # overlapping the remaining Q tiles + all of V.
                    q_fm = gp.tile([128, DK, TPC], BF16, tag="qfm")
                    k_loc = zp.tile([128, 2, DK, 128], BF16, tag="kvout")

                    def _k_write():
                        for sl_ in range(2):
                            nc.sync.dma_start(
                                out=bass.AP(tensor=kv_cat[:].tensor,
                                            offset=sl_ * SL_CAT + SL_V,
                                            ap=[[DK * 128, 128], [1, DK * 128]]),
                                in_=k_loc[:, sl_])

                    def _load_wqk(ch):
                        wt = wqkp.tile([128, DK, 512], BF16, tag="wqk")
                        nc.sync.dma_start(
                            out=wt[:],
                            in_=wqk[layer, :, ch * 512:(ch + 1) * 512]
                            .rearrange("(k p) m -> p k m", p=128))
                        return wt

                    def _qk_mms(ch, wt, pqk, mms):
                        for mm in mms:
                            m = ch * 4 + mm
                            pq = pqk.tile([128, TPC], F32)
                            for k in range(DK):
                                nc.tensor.matmul(
                                    pq[:], wt[:, k, mm * 128:(mm + 1) * 128],
                                    z_fm[:, k, :],
                                    start=(k == 0), stop=(k == DK - 1))
                            if m >= DK:
                                nc.vector.tensor_scalar_mul(
                                    q_fm[:, m - DK, :], pq[:], 1.0 / 8.0)
                            else:
                                nc.vector.tensor_copy(k_loc[:, 0, m, :],
                                                      pq[:, 0:128])
                                nc.vector.tensor_copy(k_loc[:, 1, m, :],
                                                      pq[:, 128:TPC])
                            if m == DK - 1:
                                _k_write()

                    # K tiles first, then V + both gathers, and only then the
                    # Q tiles: the gathers trigger as early as possible and
                    # the Q matmuls execute inside the gather window
                    pqk_stack = ExitStack()
                    pqk = pqk_stack.enter_context(
                        tc.tile_pool(name="pqk", bufs=4, space="PSUM"))
                    wqk0 = _load_wqk(0)
                    wqk1 = _load_wqk(1)
                    _qk_mms(0, wqk0, pqk, range(4))
                    _qk_mms(1, wqk1, pqk, (0, 1))

                    # ---- v token-major with on-chip [12x(64+1)] interleave
                    v_loc = zp.tile([128, 2, H * (HD + 1)], BF16, tag="kvout")
                    wvt = wp.tile([128, DK, D], BF16, tag="wsq")
                    nc.sync.dma_start(
                        out=wvt[:],
                        in_=wv[layer].rearrange("(k p) n -> p k n", p=128))
                    def _kv_gather(sl_):
                        # per-slot AllGather: slot 0 lands ~20us earlier than
                        # a merged gather would, and attention half 0 only
                        # needs slot-0 KV, so it starts while slot 1 flies
                        if fake_coll:
                            for R in range(4):
                                nc.gpsimd.dma_start(
                                    out=kv_all[(sl_ * 4 + R) * SL_CAT:
                                               (sl_ * 4 + R + 1) * SL_CAT],
                                    in_=kv_cat[sl_ * SL_CAT:(sl_ + 1) * SL_CAT])
                        else:
                            nc.gpsimd.collective_compute(
                                "AllGather", mybir.AluOpType.bypass,
                                replica_groups=[[0, 1, 2, 3], [4, 5, 6, 7]],
                                ins=[kv_cat[sl_ * SL_CAT:(sl_ + 1) * SL_CAT]],
                                outs=[kv_all[sl_ * 4 * SL_CAT:
                                             (sl_ + 1) * 4 * SL_CAT]])

                    with tc.tile_pool(name="pv", bufs=2, space="PSUM") as pv:
                        for s in range(2):
                            vv = v_loc[:, s, :].rearrange("p (h c) -> p h c",
                                                          c=HD + 1)
                            nc.vector.memset(vv[:, :, HD:HD + 1], 1.0)
                            for ch in range(3):  # 4 heads per 256-col chunk
                                pvt = pv.tile([128, 256], F32)
                                for k in range(DK):
                                    nc.tensor.matmul(
                                        pvt[:],
                                        z_fm[:, k, s * 128:(s + 1) * 128],
                                        wvt[:, k, ch * 256:(ch + 1) * 256],
                                        start=(k == 0), stop=(k == DK - 1))
                                if ch % 2 == 0:
                                    nc.vector.tensor_copy(
                                        vv[:, 4 * ch:4 * ch + 4, 0:HD],
                                        pvt[:].rearrange("p (h c) -> p h c", c=HD))
                                else:
                                    nc.scalar.copy(
                                        vv[:, 4 * ch:4 * ch + 4, 0:HD],
                                        pvt[:].rearrange("p (h c) -> p h c", c=HD))
                            nc.sync.dma_start(
                                out=bass.AP(tensor=kv_cat[:].tensor,
                                            offset=s * SL_CAT,
                                            ap=[[780, 128], [1, 780]]),
                                in_=v_loc[:, s, :])
                            _kv_gather(s)

                    _qk_mms(1, wqk1, pqk, (2, 3))
                    _qk_mms(2, _load_wqk(2), pqk, range(4))
                    pqk_stack.close()

                    if debug_taps and layer == 0:
                        nc.sync.dma_start(out=dbg_kv[:2 * 4 * SL_CAT], in_=kv_all[:])
                        nc.sync.dma_start(out=dbg_q[:], in_=q_fm[:])
                    if debug_taps and layer == 0:
                        post_attn_dbg = True
                    else:
                        post_attn_dbg = False

                    # ---- load gathered K then V, one batched DMA per
                    # slot.  Queue placement matters: a gather-dependent DMA
                    # parks its whole queue, so slot 0 rides the ACT queue
                    # (first exp needs it anyway) and slot 1 rides the Pool
                    # queue right behind the slot-1 collective; the sync
                    # queue carries only the weight stream and never stalls.
                    k_g, v_g = [], []
                    for sl, eng in ((0, nc.scalar), (1, nc.sync)):
                        kt = kvp.tile([128, 4, D], BF16, tag=f"kg{sl}")
                        eng.dma_start(out=kt[:], in_=bass.AP(
                            tensor=kv_all[:].tensor,
                            offset=sl * 4 * SL_CAT + SL_V,
                            ap=[[D, 128], [SL_CAT, 4], [1, D]]))
                        k_g.append(kt)
                        vt = kvp.tile([128, 4, H * (HD + 1)], BF16,
                                      tag=f"vg{sl}")
                        eng.dma_start(out=vt[:], in_=bass.AP(
                            tensor=kv_all[:].tensor,
                            offset=sl * 4 * SL_CAT,
                            ap=[[H * (HD + 1), 128], [SL_CAT, 4],
                                [1, H * (HD + 1)]]))
                        v_g.append(vt)


                    wpt = wp.tile([128, DK, D], BF16, tag="wsq")
                    nc.sync.dma_start(
                        out=wpt[:],
                        in_=wproj[layer].rearrange("(k p) n -> p k n", p=128))


                    # ---- attention, head-paired: scores for heads (2hh, 2hh+1)
                    # issue adjacently on partition rows 0-63 / 64-127 (distinct
                    # row groups -> concurrent on HW); AV is transposed
                    # (out[q, 64+1] = pt^T @ V) so the denominator lands as a
                    # per-partition column and normalize is a plain
                    # tensor_scalar on DVE.  Half 0's MLP chunks are emitted
                    # interleaved with half 1's pairs so the in-order PE
                    # stream has ready work during the softmax ACT phases.
                    y_sb = zp.tile([128, 2, D], BF16, tag="z")
                    y_fm = gp.tile([128, DK, TPC], BF16, tag="yfm")
                    z2_fm = zp.tile([128, DK, TPC], BF16, tag="zfm")
                    z2_sb = zp.tile([128, 2, D], BF16, tag="z")
                    g_fm = gp.tile([128, 24, TPC], BF16, tag="gfm")
                    with tc.tile_pool(name="pml", bufs=2, space="PSUM") as pml, \
                         ExitStack() as att_stack:
                        pstp = att_stack.enter_context(
                            tc.tile_pool(name="pat", bufs=2, space="PSUM"))
                        ppvp = att_stack.enter_context(
                            tc.tile_pool(name="ppv", bufs=2, space="PSUM"))

                        def attn_scores(half, hh):
                            hb = 4 if half == 0 else 8
                            qs = half * 128
                            pt01 = ptp.tile([128, 2, 8, 128], BF16, tag="pt")
                            pt0 = pt01[:, 0]
                            pt1 = pt01[:, 1]
                            for g4 in range(hb // 4):
                                # both heads' 4 score tiles in one 2-bank
                                # psum tile -> a single exp covers the pair
                                ps01 = pstp.tile([128, 1024], F32, tag="pst")
                                for bb_ in range(4):
                                    b = g4 * 4 + bb_
                                    for po, off in ((0, 0), (64, 512)):
                                        nc.tensor.matmul(
                                            ps01[:, off + bb_ * 128:
                                                 off + (bb_ + 1) * 128],
                                            k_g[SLOT_OF[b]][po:po + 64, RANK_OF[b],
                                                            hh * 128:(hh + 1) * 128],
                                            q_fm[po:po + 64, hh, qs:qs + 128],
                                            start=(bb_ == 0), stop=(bb_ == 3),
                                            skip_group_check=True)
                                nc.scalar.activation(
                                    pt01[:, :, g4 * 4:(g4 + 1) * 4, :],
                                    ps01[:].rearrange("p (h b i) -> p h b i",
                                                      h=2, b=4),
                                    mybir.ActivationFunctionType.Exp)
                                for pt in (pt0, pt1):
                                    nc.vector.tensor_mul(
                                        pt[:, g4 * 4:(g4 + 1) * 4, :],
                                        pt[:, g4 * 4:(g4 + 1) * 4, :],
                                        bias_sb[:, g4 * 4:(g4 + 1) * 4, qs:qs + 128])
                            return pt0, pt1

                        def attn_av(half, hh, pt0, pt1):
                            hb = 4 if half == 0 else 8
                            h0, h1 = 2 * hh, 2 * hh + 1
                            # AV pair shares one PSUM bank: h0 at cols 0:65
                            # (its start clears the bank), h1 at 128:193
                            # relying on per-element has_written
                            pv01 = ppvp.tile([128, 512], F32, tag="ppv")
                            for b in range(hb):
                                nc.tensor.matmul(
                                    pv01[:, 0:HD + 1],
                                    pt0[:, b, :],
                                    v_g[SLOT_OF[b]][:, RANK_OF[b],
                                                    h0 * (HD + 1):(h0 + 1) * (HD + 1)],
                                    start=(b == 0), stop=False,
                                    skip_group_check=True)
                                nc.tensor.matmul(
                                    pv01[:, 128:128 + HD + 1],
                                    pt1[:, b, :],
                                    v_g[SLOT_OF[b]][:, RANK_OF[b],
                                                    h1 * (HD + 1):(h1 + 1) * (HD + 1)],
                                    start=False, stop=(b == hb - 1),
                                    skip_group_check=True)
                            for off, h in ((0, h0), (128, h1)):
                                rec = work.tile([128, 1], F32, tag="srec")
                                nc.vector.reciprocal(
                                    rec[:], pv01[:, off + HD:off + HD + 1])
                                nc.vector.tensor_scalar_mul(
                                    y_sb[:, half, h * HD:(h + 1) * HD],
                                    pv01[:, off:off + HD], rec[:])

                        def _load_fc_w(ch):
                            wt = wp4.tile([128, DK, D], BF16, tag="wmlp")
                            nc.sync.dma_start(
                                out=wt[:],
                                in_=wfc[layer, :, ch * D:(ch + 1) * D]
                                .rearrange("(k p) m -> p k m", p=128))
                            return wt

                        def _load_fc2_w(ch):
                            wt = wp4.tile([128, DK, D], BF16, tag="wmlp")
                            nc.sync.dma_start(
                                out=wt[:],
                                in_=wfc2[layer, ch * D:(ch + 1) * D, :]
                                .rearrange("(k p) n -> p k n", p=128))
                            return wt

                        def y_transpose(half):
                            qs = half * 128
                            for a in range(DK):
                                ptt = pml.tile([128, 128], BF16, tag="mm")
                                nc.tensor.transpose(
                                    ptt[:], y_sb[:, half, a * 128:(a + 1) * 128],
                                    ident[:])
                                if a % 2 == 0:
                                    nc.scalar.copy(y_fm[:, a, qs:qs + 128], ptt[:])
                                else:
                                    nc.vector.tensor_copy(y_fm[:, a, qs:qs + 128],
                                                          ptt[:])
                            if post_attn_dbg and half == 1:
                                nc.sync.dma_start(out=dbg_y[:], in_=y_fm[:])

                        def proj_chunk(half, noff, nsz):
                            qs = half * 128
                            pp = pml.tile([128, 512], F32, tag="mm")
                            for k in range(DK):
                                nc.tensor.matmul(
                                    pp[:, :nsz],
                                    y_fm[:, k, qs:qs + 128],
                                    wpt[:, k, noff:noff + nsz],
                                    start=(k == 0), stop=(k == DK - 1))
                            nc.vector.tensor_add(x_sb[:, half, noff:noff + nsz],
                                                 x_sb[:, half, noff:noff + nsz],
                                                 pp[:, :nsz])

                        def ln2_chunk(half):
                            qs = half * 128
                            layer_norm_half(z2_sb, half)
                            for a in range(DK):
                                ptt = pml.tile([128, 128], BF16, tag="mm")
                                nc.tensor.transpose(
                                    ptt[:], z2_sb[:, half, a * 128:(a + 1) * 128],
                                    ident[:])
                                if a % 2 == 0:
                                    nc.scalar.copy(z2_fm[:, a, qs:qs + 128], ptt[:])
                                else:
                                    nc.vector.tensor_copy(z2_fm[:, a, qs:qs + 128],
                                                          ptt[:])

                        def fc_chunk(half, ch, wt):
                            qs = half * 128
                            for mm_ in range(6):
                                m = ch * 6 + mm_
                                pg = pml.tile([128, 128], F32, tag="mm")
                                for k in range(DK):
                                    nc.tensor.matmul(
                                        pg[:], wt[:, k, mm_ * 128:(mm_ + 1) * 128],
                                        z2_fm[:, k, qs:qs + 128],
                                        start=(k == 0), stop=(k == DK - 1))
                                nc.scalar.activation(
                                    g_fm[:, m, qs:qs + 128], pg[:],
                                    mybir.ActivationFunctionType.Gelu_apprx_tanh)

                        mlp = 'mlp' not in skip
                        fcw = [_load_fc_w(ch) for ch in range(3)] if mlp else []

                        # half-0 MLP chunks interleaved into half-1 pairs;
                        # within a pair the chunk sits between the score MMs
                        # and the exp-dependent AV MMs so the in-order PE
                        # queue always has ready work while ACT runs exp
                        # keep ACT-free work (transpose/proj) in the
                        # interleave; LN2 (sqrt) and fc (gelu) run after
                        # attention so the ACT table set switches only
                        # sqrt -> exp -> sqrt -> gelu per layer (~1.3us per
                        # table load)
                        chunks0 = [lambda: y_transpose(0),
                                   lambda: proj_chunk(0, 0, 512),
                                   lambda: proj_chunk(0, 512, 256)]
                        if 'attn' in skip:
                            nc.vector.memset(y_sb[:], 0.0)
                        else:
                            # half 0: software-pipeline scores(hh+1) ahead of
                            # AV(hh) so exp latency is hidden
                            pts = attn_scores(0, 0)
                            for hh in range(1, 6):
                                nxt = attn_scores(0, hh)
                                attn_av(0, hh - 1, *pts)
                                pts = nxt
                            attn_av(0, 5, *pts)
                        for hh in range(6):
                            if 'attn' not in skip:
                                pts = attn_scores(1, hh)
                            if hh < len(chunks0):
                                chunks0[hh]()
                            if 'attn' not in skip:
                                attn_av(1, hh, *pts)
                        for fn in chunks0[6:]:
                            fn()

                        # half-1 MLP + remaining fc chunks; weight-slot reuse
                        # order: fc(1,c0) frees t0 -> load c3; fc(1,c1) frees
                        # t1 -> load fc2w0; etc.
                        y_transpose(1)
                        proj_chunk(1, 0, 512)
                        proj_chunk(1, 512, 256)
                        fc2w = []
                        if mlp:
                            ln2_chunk(0)
                            ln2_chunk(1)
                            fc_chunk(0, 0, fcw[0])
                            fc_chunk(0, 1, fcw[1])
                            fc_chunk(0, 2, fcw[2])
                            fc_chunk(1, 0, fcw[0])
                            fcw.append(_load_fc_w(3))
                            fc_chunk(1, 1, fcw[1])
                            fc2w.append(_load_fc2_w(0))
                            fc_chunk(1, 2, fcw[2])
                            fc2w.append(_load_fc2_w(1))
                            fc_chunk(0, 3, fcw[3])
                            fc_chunk(1, 3, fcw[3])
                            fc2w.append(_load_fc2_w(2))

                        att_stack.close()
                        if mlp:
                            # ---- fc2 (token-major out, both halves) +
                            # residual.  3 banks: 512-wide chunk per half in
                            # its own bank, both 256-wide tails packed into
                            # one bank via per-element has_written (the s0
                            # start clears the bank before s1's first write)
                            with tc.tile_pool(name="pf2", bufs=1,
                                              space="PSUM") as pf2:
                                pf2_a = pf2.tile([128, 512], F32, tag="f2a")
                                pf2_b = pf2.tile([128, 512], F32, tag="f2b")
                                pf2_c = pf2.tile([128, 512], F32, tag="f2c")
                                outs = [(pf2_a[:, 0:512], 0, 0, 512, True),
                                        (pf2_c[:, 0:256], 0, 512, 256, True),
                                        (pf2_b[:, 0:512], 1, 0, 512, True),
                                        (pf2_c[:, 256:512], 1, 512, 256, False)]
                                for ch in range(4):
                                    if ch == 3:
                                        fc2w.append(_load_fc2_w(3))
                                    wt = fc2w[ch]
                                    for kk in range(DK):
                                        K24 = ch * DK + kk
                                        for po, s, noff, nsz, first in outs:
                                            nc.tensor.matmul(
                                                po,
                                                g_fm[:, K24, s * 128:(s + 1) * 128],
                                                wt[:, kk, noff:noff + nsz],
                                                start=(K24 == 0 and first),
                                                stop=(K24 == 23),
                                                skip_group_check=True)
                                for po, s, noff, nsz, first in outs:
                                    nc.vector.tensor_add(
                                        x_sb[:, s, noff:noff + nsz],
                                        x_sb[:, s, noff:noff + nsz], po)

                    if debug_taps:
                        nc.sync.dma_start(
                            out=dbg[layer + 1].rearrange("(s p) d -> p s d", p=128),
                            in_=x_sb[:])

                # ---- final LN + transpose + per-slot AllGather of hidden
                # states: slot 0 gathers while slot 1 is normalized, and the
                # slot-1 gather hides under the slot-0 lm matmuls
                z3_sb = zp.tile([128, 2, D], BF16, tag="z")
                z3_fm = zp.tile([128, DK, TPC], BF16, tag="zfm")
                for s in range(2):
                    layer_norm_half(z3_sb, s)
                    with tc.tile_pool(name=f"ptr3{s}", bufs=3,
                                      space="PSUM") as ptr3:
                        for a in range(DK):
                            pt_ = ptr3.tile([128, 128], BF16)
                            nc.tensor.transpose(
                                pt_[:], z3_sb[:, s, a * 128:(a + 1) * 128],
                                ident[:])
                            if a % 2 == 0:
                                nc.scalar.copy(
                                    z3_fm[:, a, s * 128:(s + 1) * 128], pt_[:])
                            else:
                                nc.vector.tensor_copy(
                                    z3_fm[:, a, s * 128:(s + 1) * 128], pt_[:])
                    nc.sync.dma_start(
                        out=bass.AP(tensor=xh_loc[:].tensor, offset=s * SL_K,
                                    ap=[[DK * 128, 128], [1, DK * 128]]),
                        in_=z3_fm[:, :, s * 128:(s + 1) * 128])
                    if fake_coll:
                        for R in range(NCORE):
                            nc.gpsimd.dma_start(
                                out=xh_all[(s * NCORE + R) * SL_K:
                                           (s * NCORE + R + 1) * SL_K],
                                in_=xh_loc[s * SL_K:(s + 1) * SL_K])
                    else:
                        nc.gpsimd.collective_compute(
                            "AllGather", mybir.AluOpType.bypass,
                            replica_groups=[[0, 1, 2, 3, 4, 5, 6, 7]],
                            ins=[xh_loc[s * SL_K:(s + 1) * SL_K]],
                            outs=[xh_all[s * NCORE * SL_K:
                                         (s + 1) * NCORE * SL_K]])

                # ---- lm head: logits[tok, vs] = xh^T @ Wlm, vocab-sharded;
                # s-major so slot-0 rows compute during the slot-1 gather
                trunk.close()
                if 'lm' in skip:
                    return
                obp = rctx.enter_context(tc.tile_pool(name=f"obp{rep}", bufs=2))
                with tc.tile_pool(name="plm", bufs=2, space="PSUM") as plm, \
                     tc.tile_pool(name="xtp", bufs=2, space="SBUF") as xtp:
                    for s in range(2):
                        xt_s = xtp.tile([128, NCORE, DK * 128], BF16, tag="xt")
                        nc.sync.dma_start(out=xt_s[:], in_=bass.AP(
                            tensor=xh_all[:].tensor, offset=s * NCORE * SL_K,
                            ap=[[DK * 128, 128], [SL_K, NCORE],
                                [1, DK * 128]]))
                        for R in range(NCORE):
                            gq, rq = R // 4, R % 4
                            blk = rq if s == 0 else 7 - rq
                            row = gq * T + blk * 128
                            ob = obp.tile([128, 4096], F32, tag="ob")
                            for half in range(2):
                                pl = plm.tile([128, 2048], F32)
                                for k in range(DK):
                                    for nb in range(4):
                                        nc.tensor.matmul(
                                            pl[:, nb * 512:(nb + 1) * 512],
                                            xt_s[:, R, k * 128:(k + 1) * 128],
                                            wlm_ks[k][:,
                                                   half * 2048 + nb * 512:
                                                   half * 2048 + (nb + 1) * 512],
                                            start=(k == 0), stop=(k == DK - 1))
                                nc.vector.tensor_copy(ob[:, half * 2048:half * 2048 + 1024],
                                                      pl[:, 0:1024])
                                nc.scalar.copy(ob[:, half * 2048 + 1024:(half + 1) * 2048],
                                               pl[:, 1024:2048])
                            nc.sync.dma_start(out=logits[row:row + 128, :], in_=ob[:])

        for rep in range(reps):
            _rep_body(rep)

    nc.compile()
    _NC_CACHE[key] = nc
    return nc


def _prep_in_maps(inputs):
    idx = np.asarray(inputs["idx"])
    age = np.asarray(inputs["age"], np.float32)
    wte = np.asarray(inputs["wte"], np.float32)
    wae_w = np.asarray(inputs["wae_w"], np.float32)
    ln1_w = np.asarray(inputs["ln1_w"], np.float32)
    ln2_w = np.asarray(inputs["ln2_w"], np.float32)
    lnf_w = np.asarray(inputs["lnf_w"], np.float32)
    attn_w = np.asarray(inputs["attn_w"], np.float32)
    proj_w = np.asarray(inputs["proj_w"], np.float32)
    fc_w = np.asarray(inputs["fc_w"], np.float32)
    fc2_w = np.asarray(inputs["fc2_w"], np.float32)
    for nm in ("ln1_b", "ln2_b", "lnf_b", "attn_b", "proj_b", "fc_b", "fc2_b"):
        assert not np.any(np.asarray(inputs[nm])), f"{nm} != 0 unsupported"

    bf = lambda a: np.ascontiguousarray(a).astype(NPBF16)

    # replicated weights (LN scale folded in)
    wqk_l, wv_l, wproj_l, wfc_l, wfc2_l = [], [], [], [], []
    for l in range(L):
        aw = attn_w[l] * ln1_w[l][None, :]
        # K columns first so the K AllGather can launch while Q computes
        wqk_l.append(np.concatenate([aw[D:2 * D].T, aw[:D].T], axis=1))
        wv_l.append(aw[2 * D:].T)
        wproj_l.append(proj_w[l].T)
        wfc_l.append((fc_w[l] * ln2_w[l][None, :]).T)
        wfc2_l.append(fc2_w[l].T)
    wqk_a = bf(np.stack(wqk_l))
    wv_a = bf(np.stack(wv_l))
    wproj_a = bf(np.stack(wproj_l))
    wfc_a = bf(np.stack(wfc_l))
    wfc2_a = bf(np.stack(wfc2_l))
    wae_s_a = bf(wae_w[:, 0::2].T)   # [384, 768]
    wae_c_a = bf(wae_w[:, 1::2].T)
    wlm_full = wte * lnf_w[None, :]  # [V, D]

    div = np.exp(np.arange(0, D, 2, dtype=np.float32) *
                 (-math.log(10000.0) / D))
    valid = idx > 0
    karange = np.arange(T)

    in_maps = []
    for c in range(NCORE):
        g, r = c // 4, c % 4
        jA, jB = r, 7 - r
        tok_idx = np.concatenate([np.arange(jA * 128, (jA + 1) * 128),
                                  np.arange(jB * 128, (jB + 1) * 128)])
        x_tok = wte[np.asarray(idx[g])[tok_idx]].astype(np.float32)
        ang = div[:, None] * (age[g][tok_idx][None, :] / 365.25)  # [384, 256]
        vq = valid[g][tok_idx]
        vk = valid[g]
        keep = (karange[:, None] <= tok_idx[None, :]) & (
            (vq[None, :] & vk[:, None]) |
            (~vq[None, :] & (karange[:, None] == tok_idx[None, :])))
        bias_tc = keep.astype(np.float32).astype(NPBF16)
        in_maps.append({
            "x_tok": x_tok,
            "sfm": bf(np.sin(ang)),
            "cfm": bf(np.cos(ang)),
            "bias_t": bias_tc,
            "wae_s": wae_s_a, "wae_c": wae_c_a,
            "wqk": wqk_a, "wv": wv_a, "wproj": wproj_a,
            "wfc": wfc_a, "wfc2": wfc2_a,
            "wlm": bf(wlm_full[c * VS:(c + 1) * VS].T),
        })
    return in_maps


last_results = None


def kernel(debug_taps=False, _trace=False, _tmpdir=None, **inputs):
    global last_results
    nc = _build_nc(debug_taps)
    in_maps = _prep_in_maps(inputs)
    res = run_bass_kernel_spmd(nc, in_maps, core_ids=list(range(NCORE)),
                               trace=_trace, tmpdir=_tmpdir)
    last_results = res
    out = np.empty((B, T, V), np.float32)
    for c in range(NCORE):
        out[:, :, c * VS:(c + 1) * VS] = \
            res.results[c]["logits"].reshape(B, T, VS)
    if debug_taps:
        return out, [r["dbg"] for r in res.results], \
            [r["dbg_kv"] for r in res.results], \
            [r["dbg_q"] for r in res.results], \
            [r["dbg_y"] for r in res.results]
    return out

